# revision 25
# baseline (speedup 1.0000x reference)
"""Trainium2 Bass kernel: causal multi-head attention with RoPE.

Model: B=4, L=2048, H=2048, NH=16 heads, head_dim=128.
  q = x @ Wq.T ; k = x @ Wk.T ; v = x @ Wv.T        (per-head split)
  q, k <- RoPE(q, k)
  attn = softmax(mask(q k^T / sqrt(hd)))
  out  = (attn @ v) heads-concat @ Wo.T

Sharding (8 cores): hybrid batch x tensor-parallel.  Core c handles
batch b = c//2 and heads half*8..half*8+7 with half = c%2.  Wq/Wk/Wv are
column-sharded (8 heads per core), Wo row-sharded; each core produces a
partial y[b] and the host sums the two partials per batch (the unshard
step) and concatenates batches.

Per-core dataflow (all SBUF-resident, bf16 inputs / fp32 accumulation):
  phase A: Q^T, K^T  [128d x 2048pos] per head (d-major) and V
           [128pos x 1024d] pos-major, via PE matmuls; RoPE on Q^T/K^T
           (rotate-half partition shuffle via SBUF->SBUF DMA, the
           elementwise part on DVE).
  phase B: flash-style causal attention per (head, 512-wide q chunk):
           S^T tile = K_blk^T Q_chunk (PE), P = exp(S^T/sqrt(d)) (ACT),
           block-sparse causal structure with a triangular-mask multiply
           on diagonal blocks (DVE), O^T += V_blk P (PE), rowsum via
           ones-matmul (PE), reciprocal+broadcast+scale for the softmax
           normalization (DVE + GPSIMD).
  phase C: y^T partial = Wo_shard O^T (PE) -> DRAM fp32.
"""

import math
import numpy as np

B, L, H, NH, HD = 4, 2048, 2048, 16, 128
ROPE_BASE = 10000.0
NCORES = 8
HPC = 8          # heads per core
QC = 512         # q chunk width
NQC = L // QC    # 4 q chunks
NKB = L // 128   # 16 kp blocks
SCALE = 1.0 / math.sqrt(HD)

_cache = {}


def _analyze_mask(mask2d):
    """Classify each (q_block, kp_block) 128x128 block of the [L, L] mask.

    Returns (block_kind[16][16] with 0=empty,1=full,2=mixed, patterns,
    pattern_idx dict keyed by block coords). mask2d is int32 [L, L],
    rows=q, cols=kp.
    """
    nb = L // 128
    kind = [[0] * nb for _ in range(nb)]
    patterns = []
    pat_key_to_idx = {}
    block_pat = {}
    for qb in range(nb):
        rows = mask2d[qb * 128:(qb + 1) * 128]
        for kb in range(nb):
            blk = rows[:, kb * 128:(kb + 1) * 128]
            s = int(blk.sum())
            if s == 0:
                kind[qb][kb] = 0
            elif s == 128 * 128:
                kind[qb][kb] = 1
            else:
                kind[qb][kb] = 2
                key = blk.tobytes()
                idx = pat_key_to_idx.get(key)
                if idx is None:
                    idx = len(patterns)
                    pat_key_to_idx[key] = idx
                    # stored transposed: S^T tiles are [kp, q]
                    patterns.append(np.ascontiguousarray(blk.T))
                block_pat[(qb, kb)] = idx
    return kind, patterns, block_pat


def _build(kind, block_pat, n_patterns):
    """Build the SPMD bass program (same for all 8 cores)."""
    import concourse.bass as bass
    import concourse.bacc as bacc
    import concourse.mybir as mybir
    import concourse.tile as tile

    fp32 = mybir.dt.float32
    bf16 = mybir.dt.bfloat16
    EXP = mybir.ActivationFunctionType.Exp

    nc = bacc.Bacc("TRN2", target_bir_lowering=False, debug=False)

    xT = nc.dram_tensor("xT", [H, L], bf16, kind="ExternalInput")
    wqT = nc.dram_tensor("wqT", [H, HPC * HD], bf16, kind="ExternalInput")
    wkT = nc.dram_tensor("wkT", [H, HPC * HD], bf16, kind="ExternalInput")
    wvT = nc.dram_tensor("wvT", [H, HPC * HD], bf16, kind="ExternalInput")
    woT = nc.dram_tensor("woT", [HPC * HD, H], bf16, kind="ExternalInput")
    cosd = nc.dram_tensor("cosd", [HD, L], bf16, kind="ExternalInput")
    sinmd = nc.dram_tensor("sinmd", [HD, L], bf16, kind="ExternalInput")
    npat = max(n_patterns, 1)
    maskd = nc.dram_tensor("maskd", [npat, 128, 128], bf16, kind="ExternalInput")
    yT = nc.dram_tensor("yT", [H, L], fp32, kind="ExternalOutput")

    NHC = H // 128  # 16 input-feature blocks

    def qk_phase(tc, w_dram, out_a, wpool, xpool, tpool, pspool, wtag,
                 cos_sb, sinm_sb):
        """Q^T / K^T d-major projection + fused RoPE per (head, chunk)."""
        w_sb = wpool.tile([128, NHC, HPC * HD], bf16, tag="w",
                          name=f"w_{wtag}")
        nc.sync.dma_start(
            out=w_sb[:], in_=w_dram[:].rearrange("(a p) m -> p a m", p=128))
        for j in range(NQC):
            js = slice(j * QC, (j + 1) * QC)
            x_sb = xpool.tile([128, NHC, QC], bf16, tag="xcols",
                              name=f"x_{wtag}{j}")
            nc.sync.dma_start(
                out=x_sb[:],
                in_=xT[:, js].rearrange("(a p) m -> p a m", p=128))
            for h in range(HPC):
                ps = pspool.tile([128, QC], fp32, tag="ps_proj")
                for hc in range(NHC):
                    nc.tensor.matmul(
                        ps[:],
                        w_sb[:, hc, h * HD:(h + 1) * HD],
                        x_sb[:, hc, :],
                        start=(hc == 0), stop=(hc == NHC - 1))
                q = out_a[:, h, js]
                nc.scalar.copy(q, ps[:])
                # rotate-half: pure partition swap, done by SBUF->SBUF DMA
                rq = tpool.tile([128, QC], bf16, tag="rotq")
                nc.sync.dma_start(out=rq[0:64, :], in_=out_a[64:128, h, js])
                nc.sync.dma_start(out=rq[64:128, :], in_=out_a[0:64, h, js])
                nc.vector.tensor_mul(rq[:], rq[:], sinm_sb[:, js])
                nc.vector.tensor_mul(q, q, cos_sb[:, js])
                nc.vector.tensor_add(q, q, rq[:])

    def v_phase(tc, w_dram, va, wpool, xpool, pspool):
        """V pos-major projection (x chunks 256 wide to fit SBUF)."""
        w_sb = wpool.tile([128, NHC, HPC * HD], bf16, tag="w", name="w_v")
        nc.sync.dma_start(
            out=w_sb[:], in_=w_dram[:].rearrange("(a p) m -> p a m", p=128))
        VC = 256
        for j in range(L // VC):
            x_sb = xpool.tile([128, NHC, VC], bf16, tag="xv", name=f"xv{j}")
            nc.sync.dma_start(
                out=x_sb[:],
                in_=xT[:, j * VC:(j + 1) * VC].rearrange(
                    "(a p) m -> p a m", p=128))
            for pb in range(VC // 128):
                for dc in range(2):
                    ps = pspool.tile([128, QC], fp32, tag="ps_proj")
                    for hc in range(NHC):
                        nc.tensor.matmul(
                            ps[:],
                            x_sb[:, hc, pb * 128:(pb + 1) * 128],
                            w_sb[:, hc, dc * QC:(dc + 1) * QC],
                            start=(hc == 0), stop=(hc == NHC - 1))
                    nc.scalar.copy(
                        va[:, j * (VC // 128) + pb, dc * QC:(dc + 1) * QC],
                        ps[:])

    with tile.TileContext(nc) as tc:
        with tc.tile_pool(name="persist", bufs=1, side="left") as persist:
            # one combined small-constant tile: [trimask patterns | ones]
            cst = persist.tile([128, npat * 128 + 128], bf16, tag="cst")
            for p in range(n_patterns):
                nc.sync.dma_start(out=cst[:, p * 128:(p + 1) * 128],
                                  in_=maskd[p])
            ones_col = npat * 128
            nc.vector.memset(cst[:, ones_col:ones_col + 128], 1.0)
            onesf = persist.tile([128, 128], fp32, tag="onesf")
            nc.vector.memset(onesf[:], 1.0)
            QTa = persist.tile([HD, HPC, L], bf16, tag="qta")
            KTa = persist.tile([HD, HPC, L], bf16, tag="kta")

            # ---------------- phase A: projections + RoPE ----------------
            # Manual pool lifetimes (non-LIFO): weights/x/rope tables are
            # freed before attention while Va spans V-phase..attention.
            wpool_cm = tc.tile_pool(name="wpool", bufs=2, side="right")
            wpool = wpool_cm.__enter__()
            ropec_cm = tc.tile_pool(name="ropec", bufs=1, side="right")
            ropec = ropec_cm.__enter__()
            psp_cm = tc.tile_pool(name="ps_proj", bufs=4, space="PSUM")
            psp = psp_cm.__enter__()

            cos_sb = ropec.tile([HD, L], bf16, tag="cos")
            sinm_sb = ropec.tile([HD, L], bf16, tag="sinm")
            nc.sync.dma_start(out=cos_sb[:], in_=cosd[:])
            nc.sync.dma_start(out=sinm_sb[:], in_=sinmd[:])

            xqk_cm = tc.tile_pool(name="xqk", bufs=2, side="right")
            xqk = xqk_cm.__enter__()
            tpool_cm = tc.tile_pool(name="tpool", bufs=3, side="right")
            tpool = tpool_cm.__enter__()
            qk_phase(tc, wqT, QTa, wpool, xqk, tpool, psp, "q",
                     cos_sb, sinm_sb)
            qk_phase(tc, wkT, KTa, wpool, xqk, tpool, psp, "k",
                     cos_sb, sinm_sb)
            tpool_cm.__exit__(None, None, None)
            xqk_cm.__exit__(None, None, None)
            ropec_cm.__exit__(None, None, None)

            vp_cm = tc.tile_pool(name="vp", bufs=1, side="left")
            vp_outer = vp_cm.__enter__()
            Va = vp_outer.tile([128, NKB, HPC * HD], bf16, tag="va")
            xv_cm = tc.tile_pool(name="xv", bufs=2, side="right")
            xv = xv_cm.__enter__()
            v_phase(tc, wvT, Va, wpool, xv, psp)
            xv_cm.__exit__(None, None, None)
            wpool_cm.__exit__(None, None, None)
            psp_cm.__exit__(None, None, None)

            # -------- phase B + C under Va's lifetime --------
            _attn_and_out(tc, nc, kind, block_pat, QTa, KTa, Va,
                          cst, ones_col, onesf, woT, yT, fp32, bf16, EXP)
            vp_cm.__exit__(None, None, None)

    nc.compile()
    return nc


def _attn_and_out(tc, nc, kind, block_pat, QTa, KTa, Va, cst, ones_col,
                  onesf, woT, yT, fp32, bf16, EXP):
    ones_sb = cst[:, ones_col:ones_col + 1]
    with tc.tile_pool(name="otp", bufs=1, side="left") as otp, \
         tc.tile_pool(name="wo", bufs=1, side="left") as wop:
        OTa = otp.tile([HD, HPC, L], bf16, tag="ota")
        wo_sb = wop.tile([128, HPC, H], bf16, tag="wo")
        # prefetch Wo during attention
        nc.sync.dma_start(
            out=wo_sb[:], in_=woT[:].rearrange("(a p) m -> p a m", p=128))

        # ---------------- phase B: attention ----------------
        # per-(head, q-chunk) flash loop; softmax normalization broadcast
        # via K=1 PE outer product + ACT copy (keeps gpsimd free)
        with tc.tile_pool(name="pp", bufs=4, side="right") as ppool, \
             tc.tile_pool(name="rr", bufs=2, side="right") as rpool, \
             tc.tile_pool(name="bb", bufs=3, side="right") as bpool, \
             tc.tile_pool(name="ps_s", bufs=3, space="PSUM") as ps_s, \
             tc.tile_pool(name="ps_o", bufs=2, space="PSUM") as ps_o, \
             tc.tile_pool(name="ps_r", bufs=2, space="PSUM") as ps_r:
            for h in range(HPC):
                for j in range(NQC):
                    blocks = []
                    for i in range(NKB):
                        live = [t for t in range(4)
                                if kind[4 * j + t][i] != 0]
                        if live:
                            blocks.append((i, live))
                    if not blocks:
                        continue
                    pso = ps_o.tile([128, QC], fp32, tag="pso",
                                    name=f"pso{h}_{j}")
                    psr = ps_r.tile([128, QC], fp32, tag="psr",
                                    name=f"psr{h}_{j}")
                    last = len(blocks) - 1
                    for bi, (i, live) in enumerate(blocks):
                        t0, t1 = live[0], live[-1]
                        w0, w1 = t0 * 128, (t1 + 1) * 128
                        pss = ps_s.tile([128, QC], fp32, tag="pss",
                                        name=f"pss{h}_{j}_{i}")
                        nc.tensor.matmul(
                            pss[:, w0:w1],
                            KTa[:, h, i * 128:(i + 1) * 128],
                            QTa[:, h, j * QC + w0:j * QC + w1],
                            start=True, stop=True)
                        P = ppool.tile([128, QC], bf16, tag="p",
                                       name=f"p{h}_{j}_{i}")
                        first = (bi == 0)
                        if w0 > 0 and first:
                            nc.vector.memset(P[:, 0:w0], 0.0)
                        if w1 < QC and first:
                            nc.vector.memset(P[:, w1:QC], 0.0)
                        nc.scalar.activation(P[:, w0:w1], pss[:, w0:w1],
                                             EXP, scale=SCALE)
                        for t in range(t0, t1 + 1):
                            qb = 4 * j + t
                            if kind[qb][i] == 0:
                                nc.vector.memset(
                                    P[:, t * 128:(t + 1) * 128], 0.0)
                            elif kind[qb][i] == 2:
                                pat = block_pat[(qb, i)]
                                nc.vector.tensor_mul(
                                    P[:, t * 128:(t + 1) * 128],
                                    P[:, t * 128:(t + 1) * 128],
                                    cst[:, pat * 128:(pat + 1) * 128])
                        m0 = 0 if first else w0
                        nc.tensor.matmul(
                            pso[:, m0:QC],
                            Va[:, i, h * HD:(h + 1) * HD],
                            P[:, m0:QC],
                            start=first, stop=(bi == last))
                        nc.tensor.matmul(
                            psr[0:1, m0:QC], ones_sb, P[:, m0:QC],
                            start=first, stop=(bi == last))
                    r_sb = rpool.tile([128, QC], fp32, tag="r",
                                      name=f"r{h}_{j}")
                    nc.vector.reciprocal_approx_fast(
                        out=r_sb[0:1, :], in_=psr[0:1, :])
                    psb = ps_s.tile([128, QC], fp32, tag="pss",
                                    name=f"psb{h}_{j}")
                    nc.tensor.matmul(psb[:], onesf[0:1, :], r_sb[0:1, :],
                                     start=True, stop=True)
                    bc_sb = bpool.tile([128, QC], fp32, tag="bc",
                                       name=f"bc{h}_{j}")
                    nc.scalar.copy(bc_sb[:], psb[:])
                    nc.vector.tensor_mul(
                        OTa[:, h, j * QC:(j + 1) * QC], pso[:], bc_sb[:])

        # ---------------- phase C: output projection ----------------
        with tc.tile_pool(name="ysb", bufs=3, side="right") as ypool, \
             tc.tile_pool(name="ps_c", bufs=4, space="PSUM") as ps_c:
            for oc in range(H // 128):
                for j in range(NQC):
                    ps = ps_c.tile([128, QC], fp32, tag="psc")
                    for fc in range(HPC):
                        nc.tensor.matmul(
                            ps[:],
                            wo_sb[:, fc, oc * 128:(oc + 1) * 128],
                            OTa[:, fc, j * QC:(j + 1) * QC],
                            start=(fc == 0), stop=(fc == HPC - 1))
                    y_sb = ypool.tile([128, QC], fp32, tag="y")
                    nc.vector.tensor_copy(y_sb[:], ps[:])
                    nc.sync.dma_start(
                        out=yT[oc * 128:(oc + 1) * 128,
                               j * QC:(j + 1) * QC],
                        in_=y_sb[:])


def _prep_inputs(x, mask, Wq, Wk, Wv, Wo, patterns):
    import ml_dtypes
    bf16 = ml_dtypes.bfloat16

    # RoPE tables, d-major [HD, L]
    inv_freq = 1.0 / (ROPE_BASE ** (np.arange(0, HD, 2, dtype=np.float64)
                                    / HD))
    t = np.arange(L, dtype=np.float64)
    freqs = np.outer(t, inv_freq)                     # [L, HD/2]
    emb = np.concatenate((freqs, freqs), axis=-1)     # [L, HD]
    cos = np.cos(emb).T.astype(np.float32)            # [HD, L]
    sin = np.sin(emb).T.astype(np.float32)
    sinm = sin.copy()
    sinm[0:64] = -sin[0:64]
    cos_b = cos.astype(bf16)
    sinm_b = sinm.astype(bf16)

    npat = max(len(patterns), 1)
    maskd = np.zeros((npat, 128, 128), dtype=bf16)
    for i, p in enumerate(patterns):
        maskd[i] = p.astype(np.float32).astype(bf16)

    in_maps = []
    for c in range(NCORES):
        b, half = c // 2, c % 2
        rows = slice(half * HPC * HD, (half + 1) * HPC * HD)
        in_maps.append({
            "xT": np.ascontiguousarray(x[b].T).astype(bf16),
            "wqT": np.ascontiguousarray(Wq[rows, :].T).astype(bf16),
            "wkT": np.ascontiguousarray(Wk[rows, :].T).astype(bf16),
            "wvT": np.ascontiguousarray(Wv[rows, :].T).astype(bf16),
            "woT": np.ascontiguousarray(Wo[:, rows].T).astype(bf16),
            "cosd": cos_b,
            "sinmd": sinm_b,
            "maskd": maskd,
        })
    return in_maps


def kernel(x, mask, Wq, Wk, Wv, Wo, _trace=False):
    from concourse.bass_utils import run_bass_kernel_spmd

    x = np.asarray(x, dtype=np.float32)
    mask2d = np.asarray(mask, dtype=np.int32).reshape(L, L)
    key = mask2d.tobytes()
    if key not in _cache:
        kind, patterns, block_pat = _analyze_mask(mask2d)
        nc = _build(kind, block_pat, len(patterns))
        _cache[key] = (nc, patterns)
    nc, patterns = _cache[key]

    in_maps = _prep_inputs(x, mask, np.asarray(Wq, np.float32),
                           np.asarray(Wk, np.float32),
                           np.asarray(Wv, np.float32),
                           np.asarray(Wo, np.float32), patterns)
    res = run_bass_kernel_spmd(nc, in_maps, list(range(NCORES)),
                               trace=_trace)
    y = np.empty((B, L, H), dtype=np.float32)
    for b in range(B):
        acc = res.results[2 * b]["yT"].astype(np.float32) + \
              res.results[2 * b + 1]["yT"].astype(np.float32)
        y[b] = acc.T
    if _trace:
        kernel.last_results = res
    return y


if __name__ == "__main__":
    import reference
    inputs = reference.setup_inputs()
    inputs = {k: np.asarray(v) for k, v in inputs.items()}
    out = kernel(**inputs)
    exp = np.asarray(reference.reference(**{k: v for k, v in inputs.items()}))
    err = np.abs(out - exp).max() / np.abs(exp).max()
    print("rel err (absmax):", err)


# revision 26
# speedup vs baseline: 1.1472x; 1.1472x over previous
"""Trainium2 Bass kernel: causal multi-head attention with RoPE.

Model: B=4, L=2048, H=2048, NH=16 heads, head_dim=128.
  q = x @ Wq.T ; k = x @ Wk.T ; v = x @ Wv.T        (per-head split)
  q, k <- RoPE(q, k)
  attn = softmax(mask(q k^T / sqrt(hd)))
  out  = (attn @ v) heads-concat @ Wo.T

Sharding (8 cores): hybrid batch x tensor-parallel.  Core c handles
batch b = c//2 and heads half*8..half*8+7 with half = c%2.  Wq/Wk/Wv are
column-sharded (8 heads per core), Wo row-sharded; each core produces a
partial y[b] and the host sums the two partials per batch (the unshard
step) and concatenates batches.

Per-core dataflow (all SBUF-resident, bf16 inputs / fp32 accumulation):
  phase A: Q^T, K^T  [128d x 2048pos] per head (d-major) and V
           [128pos x 1024d] pos-major, via PE matmuls; RoPE on Q^T/K^T
           (rotate-half partition shuffle via SBUF->SBUF DMA, the
           elementwise part on DVE).
  phase B: flash-style causal attention per (head, 512-wide q chunk):
           S^T tile = K_blk^T Q_chunk (PE), P = exp(S^T/sqrt(d)) (ACT),
           block-sparse causal structure with a triangular-mask multiply
           on diagonal blocks (DVE), O^T += V_blk P (PE), rowsum via
           ones-matmul (PE), reciprocal+broadcast+scale for the softmax
           normalization (DVE + GPSIMD).
  phase C: y^T partial = Wo_shard O^T (PE) -> DRAM fp32.
"""

import math
import numpy as np

B, L, H, NH, HD = 4, 2048, 2048, 16, 128
ROPE_BASE = 10000.0
NCORES = 8
HPC = 8          # heads per core
QC = 512         # q chunk width
NQC = L // QC    # 4 q chunks
NKB = L // 128   # 16 kp blocks
SCALE = 1.0 / math.sqrt(HD)

_cache = {}


def _analyze_mask(mask2d):
    """Classify each (q_block, kp_block) 128x128 block of the [L, L] mask.

    Returns (block_kind[16][16] with 0=empty,1=full,2=mixed, patterns,
    pattern_idx dict keyed by block coords). mask2d is int32 [L, L],
    rows=q, cols=kp.
    """
    nb = L // 128
    kind = [[0] * nb for _ in range(nb)]
    patterns = []
    pat_key_to_idx = {}
    block_pat = {}
    for qb in range(nb):
        rows = mask2d[qb * 128:(qb + 1) * 128]
        for kb in range(nb):
            blk = rows[:, kb * 128:(kb + 1) * 128]
            s = int(blk.sum())
            if s == 0:
                kind[qb][kb] = 0
            elif s == 128 * 128:
                kind[qb][kb] = 1
            else:
                kind[qb][kb] = 2
                key = blk.tobytes()
                idx = pat_key_to_idx.get(key)
                if idx is None:
                    idx = len(patterns)
                    pat_key_to_idx[key] = idx
                    # stored transposed: S^T tiles are [kp, q]
                    patterns.append(np.ascontiguousarray(blk.T))
                block_pat[(qb, kb)] = idx
    return kind, patterns, block_pat


def _build(kind, block_pat, n_patterns):
    """Build the SPMD bass program (same for all 8 cores)."""
    import concourse.bass as bass
    import concourse.bacc as bacc
    import concourse.mybir as mybir
    import concourse.tile as tile

    fp32 = mybir.dt.float32
    bf16 = mybir.dt.bfloat16
    EXP = mybir.ActivationFunctionType.Exp

    nc = bacc.Bacc("TRN2", target_bir_lowering=False, debug=False)

    xT = nc.dram_tensor("xT", [H, L], bf16, kind="ExternalInput")
    wqT = nc.dram_tensor("wqT", [H, HPC * HD], bf16, kind="ExternalInput")
    wkT = nc.dram_tensor("wkT", [H, HPC * HD], bf16, kind="ExternalInput")
    wvT = nc.dram_tensor("wvT", [H, HPC * HD], bf16, kind="ExternalInput")
    woT = nc.dram_tensor("woT", [HPC * HD, H], bf16, kind="ExternalInput")
    cosd = nc.dram_tensor("cosd", [HD, L], bf16, kind="ExternalInput")
    sinmd = nc.dram_tensor("sinmd", [HD, L], bf16, kind="ExternalInput")
    npat = max(n_patterns, 1)
    maskd = nc.dram_tensor("maskd", [npat, 128, 128], bf16, kind="ExternalInput")
    yT = nc.dram_tensor("yT", [H, L], fp32, kind="ExternalOutput")

    NHC = H // 128  # 16 input-feature blocks

    def qk_phase(tc, w_dram, out_a, wpool, xpool, tpool, pspool, wtag,
                 cos_sb, sinm_sb):
        """Q^T / K^T d-major projection + fused RoPE per (head, chunk)."""
        w_sb = wpool.tile([128, NHC, HPC * HD], bf16, tag="w",
                          name=f"w_{wtag}")
        wr = w_dram[:].rearrange("(a p) m -> p a m", p=128)
        nc.sync.dma_start(out=w_sb[:, 0:4, :], in_=wr[:, 0:4, :])
        nc.sync.dma_start(out=w_sb[:, 4:8, :], in_=wr[:, 4:8, :])
        nc.sync.dma_start(out=w_sb[:, 8:12, :], in_=wr[:, 8:12, :])
        nc.sync.dma_start(out=w_sb[:, 12:16, :], in_=wr[:, 12:16, :])
        for j in range(NQC):
            js = slice(j * QC, (j + 1) * QC)
            x_sb = xpool.tile([128, NHC, QC], bf16, tag="xcols",
                              name=f"x_{wtag}{j}")
            xr = xT[:, js].rearrange("(a p) m -> p a m", p=128)
            nc.sync.dma_start(out=x_sb[:, 0:4, :], in_=xr[:, 0:4, :])
            nc.sync.dma_start(out=x_sb[:, 4:8, :], in_=xr[:, 4:8, :])
            nc.sync.dma_start(out=x_sb[:, 8:12, :], in_=xr[:, 8:12, :])
            nc.sync.dma_start(out=x_sb[:, 12:16, :], in_=xr[:, 12:16, :])
            for h in range(HPC):
                ps = pspool.tile([128, QC], fp32, tag="ps_proj")
                for hc in range(NHC):
                    nc.tensor.matmul(
                        ps[:],
                        w_sb[:, hc, h * HD:(h + 1) * HD],
                        x_sb[:, hc, :],
                        start=(hc == 0), stop=(hc == NHC - 1))
                q = out_a[:, h, js]
                nc.scalar.copy(q, ps[:])
                # rotate-half: pure partition swap, done by SBUF->SBUF DMA
                rq = tpool.tile([128, QC], bf16, tag="rotq")
                nc.sync.dma_start(out=rq[0:64, :], in_=out_a[64:128, h, js])
                nc.sync.dma_start(out=rq[64:128, :], in_=out_a[0:64, h, js])
                nc.vector.tensor_mul(rq[:], rq[:], sinm_sb[:, js])
                nc.vector.tensor_mul(q, q, cos_sb[:, js])
                nc.vector.tensor_add(q, q, rq[:])

    def v_phase(tc, w_dram, va, wpool, xpool, pspool):
        """V pos-major projection (x chunks 256 wide to fit SBUF)."""
        w_sb = wpool.tile([128, NHC, HPC * HD], bf16, tag="w", name="w_v")
        nc.sync.dma_start(
            out=w_sb[:], in_=w_dram[:].rearrange("(a p) m -> p a m", p=128))
        VC = 256
        for j in range(L // VC):
            x_sb = xpool.tile([128, NHC, VC], bf16, tag="xv", name=f"xv{j}")
            nc.sync.dma_start(
                out=x_sb[:],
                in_=xT[:, j * VC:(j + 1) * VC].rearrange(
                    "(a p) m -> p a m", p=128))
            for pb in range(VC // 128):
                for dc in range(2):
                    ps = pspool.tile([128, QC], fp32, tag="ps_proj")
                    for hc in range(NHC):
                        nc.tensor.matmul(
                            ps[:],
                            x_sb[:, hc, pb * 128:(pb + 1) * 128],
                            w_sb[:, hc, dc * QC:(dc + 1) * QC],
                            start=(hc == 0), stop=(hc == NHC - 1))
                    nc.scalar.copy(
                        va[:, j * (VC // 128) + pb, dc * QC:(dc + 1) * QC],
                        ps[:])

    with tile.TileContext(nc) as tc:
        with tc.tile_pool(name="persist", bufs=1, side="left") as persist:
            # one combined small-constant tile: [trimask patterns | ones]
            cst = persist.tile([128, npat * 128 + 128], bf16, tag="cst")
            for p in range(n_patterns):
                nc.sync.dma_start(out=cst[:, p * 128:(p + 1) * 128],
                                  in_=maskd[p])
            ones_col = npat * 128
            nc.vector.memset(cst[:, ones_col:ones_col + 128], 1.0)
            onesf = persist.tile([128, 128], fp32, tag="onesf")
            nc.vector.memset(onesf[:], 1.0)
            QTa = persist.tile([HD, HPC, L], bf16, tag="qta")
            KTa = persist.tile([HD, HPC, L], bf16, tag="kta")

            # ---------------- phase A: projections + RoPE ----------------
            # Manual pool lifetimes (non-LIFO): weights/x/rope tables are
            # freed before attention while Va spans V-phase..attention.
            wpool_cm = tc.tile_pool(name="wpool", bufs=2, side="right")
            wpool = wpool_cm.__enter__()
            ropec_cm = tc.tile_pool(name="ropec", bufs=1, side="right")
            ropec = ropec_cm.__enter__()
            psp_cm = tc.tile_pool(name="ps_proj", bufs=4, space="PSUM")
            psp = psp_cm.__enter__()

            cos_sb = ropec.tile([HD, L], bf16, tag="cos")
            sinm_sb = ropec.tile([HD, L], bf16, tag="sinm")
            nc.sync.dma_start(out=cos_sb[:], in_=cosd[:])
            nc.sync.dma_start(out=sinm_sb[:], in_=sinmd[:])

            xqk_cm = tc.tile_pool(name="xqk", bufs=2, side="right")
            xqk = xqk_cm.__enter__()
            tpool_cm = tc.tile_pool(name="tpool", bufs=3, side="right")
            tpool = tpool_cm.__enter__()
            qk_phase(tc, wqT, QTa, wpool, xqk, tpool, psp, "q",
                     cos_sb, sinm_sb)
            qk_phase(tc, wkT, KTa, wpool, xqk, tpool, psp, "k",
                     cos_sb, sinm_sb)
            tpool_cm.__exit__(None, None, None)
            xqk_cm.__exit__(None, None, None)
            ropec_cm.__exit__(None, None, None)

            vp_cm = tc.tile_pool(name="vp", bufs=1, side="left")
            vp_outer = vp_cm.__enter__()
            Va = vp_outer.tile([128, NKB, HPC * HD], bf16, tag="va")
            xv_cm = tc.tile_pool(name="xv", bufs=2, side="right")
            xv = xv_cm.__enter__()
            v_phase(tc, wvT, Va, wpool, xv, psp)
            xv_cm.__exit__(None, None, None)
            wpool_cm.__exit__(None, None, None)
            psp_cm.__exit__(None, None, None)

            # -------- phase B + C under Va's lifetime --------
            _attn_and_out(tc, nc, kind, block_pat, QTa, KTa, Va,
                          cst, ones_col, onesf, woT, yT, fp32, bf16, EXP)
            vp_cm.__exit__(None, None, None)

    nc.compile()
    return nc


def _attn_and_out(tc, nc, kind, block_pat, QTa, KTa, Va, cst, ones_col,
                  onesf, woT, yT, fp32, bf16, EXP):
    ones_sb = cst[:, ones_col:ones_col + 1]
    with tc.tile_pool(name="otp", bufs=1, side="left") as otp, \
         tc.tile_pool(name="wo", bufs=1, side="left") as wop:
        OTa = otp.tile([HD, HPC, L], bf16, tag="ota")
        wo_sb = wop.tile([128, HPC, H], bf16, tag="wo")
        # prefetch Wo during attention
        nc.sync.dma_start(
            out=wo_sb[:], in_=woT[:].rearrange("(a p) m -> p a m", p=128))

        # ---------------- phase B: attention ----------------
        # per-(head, q-chunk) flash loop; softmax normalization broadcast
        # via K=1 PE outer product + ACT copy (keeps gpsimd free)
        with tc.tile_pool(name="pp", bufs=4, side="right") as ppool, \
             tc.tile_pool(name="rr", bufs=2, side="right") as rpool, \
             tc.tile_pool(name="bb", bufs=3, side="right") as bpool, \
             tc.tile_pool(name="ps_s", bufs=3, space="PSUM") as ps_s, \
             tc.tile_pool(name="ps_o", bufs=2, space="PSUM") as ps_o, \
             tc.tile_pool(name="ps_r", bufs=2, space="PSUM") as ps_r:
            for h in range(HPC):
                for j in range(NQC):
                    blocks = []
                    for i in range(NKB):
                        live = [t for t in range(4)
                                if kind[4 * j + t][i] != 0]
                        if live:
                            blocks.append((i, live))
                    if not blocks:
                        continue
                    pso = ps_o.tile([128, QC], fp32, tag="pso",
                                    name=f"pso{h}_{j}")
                    psr = ps_r.tile([128, QC], fp32, tag="psr",
                                    name=f"psr{h}_{j}")
                    last = len(blocks) - 1
                    for bi, (i, live) in enumerate(blocks):
                        t0, t1 = live[0], live[-1]
                        w0, w1 = t0 * 128, (t1 + 1) * 128
                        pss = ps_s.tile([128, QC], fp32, tag="pss",
                                        name=f"pss{h}_{j}_{i}")
                        nc.tensor.matmul(
                            pss[:, w0:w1],
                            KTa[:, h, i * 128:(i + 1) * 128],
                            QTa[:, h, j * QC + w0:j * QC + w1],
                            start=True, stop=True)
                        P = ppool.tile([128, QC], bf16, tag="p",
                                       name=f"p{h}_{j}_{i}")
                        first = (bi == 0)
                        if w0 > 0 and first:
                            nc.vector.memset(P[:, 0:w0], 0.0)
                        if w1 < QC and first:
                            nc.vector.memset(P[:, w1:QC], 0.0)
                        nc.scalar.activation(P[:, w0:w1], pss[:, w0:w1],
                                             EXP, scale=SCALE)
                        for t in range(t0, t1 + 1):
                            qb = 4 * j + t
                            if kind[qb][i] == 0:
                                nc.vector.memset(
                                    P[:, t * 128:(t + 1) * 128], 0.0)
                            elif kind[qb][i] == 2:
                                pat = block_pat[(qb, i)]
                                nc.vector.tensor_mul(
                                    P[:, t * 128:(t + 1) * 128],
                                    P[:, t * 128:(t + 1) * 128],
                                    cst[:, pat * 128:(pat + 1) * 128])
                        m0 = 0 if first else w0
                        nc.tensor.matmul(
                            pso[:, m0:QC],
                            Va[:, i, h * HD:(h + 1) * HD],
                            P[:, m0:QC],
                            start=first, stop=(bi == last))
                        nc.tensor.matmul(
                            psr[0:1, m0:QC], ones_sb, P[:, m0:QC],
                            start=first, stop=(bi == last))
                    r_sb = rpool.tile([128, QC], fp32, tag="r",
                                      name=f"r{h}_{j}")
                    nc.vector.reciprocal_approx_fast(
                        out=r_sb[0:1, :], in_=psr[0:1, :])
                    rb_sb = rpool.tile([128, QC], bf16, tag="rb",
                                       name=f"rb{h}_{j}")
                    nc.vector.tensor_copy(rb_sb[0:1, :], r_sb[0:1, :])
                    bc_sb = bpool.tile([128, QC], bf16, tag="bc",
                                       name=f"bc{h}_{j}")
                    nc.gpsimd.partition_broadcast(bc_sb[:], rb_sb[0:1, :])
                    nc.vector.tensor_mul(
                        OTa[:, h, j * QC:(j + 1) * QC], pso[:], bc_sb[:])

        # ---------------- phase C: output projection ----------------
        with tc.tile_pool(name="ysb", bufs=3, side="right") as ypool, \
             tc.tile_pool(name="ps_c", bufs=4, space="PSUM") as ps_c:
            for oc in range(H // 128):
                for j in range(NQC):
                    ps = ps_c.tile([128, QC], fp32, tag="psc")
                    for fc in range(HPC):
                        nc.tensor.matmul(
                            ps[:],
                            wo_sb[:, fc, oc * 128:(oc + 1) * 128],
                            OTa[:, fc, j * QC:(j + 1) * QC],
                            start=(fc == 0), stop=(fc == HPC - 1))
                    y_sb = ypool.tile([128, QC], fp32, tag="y")
                    nc.vector.tensor_copy(y_sb[:], ps[:])
                    nc.sync.dma_start(
                        out=yT[oc * 128:(oc + 1) * 128,
                               j * QC:(j + 1) * QC],
                        in_=y_sb[:])


def _prep_inputs(x, mask, Wq, Wk, Wv, Wo, patterns):
    import ml_dtypes
    bf16 = ml_dtypes.bfloat16

    # RoPE tables, d-major [HD, L]
    inv_freq = 1.0 / (ROPE_BASE ** (np.arange(0, HD, 2, dtype=np.float64)
                                    / HD))
    t = np.arange(L, dtype=np.float64)
    freqs = np.outer(t, inv_freq)                     # [L, HD/2]
    emb = np.concatenate((freqs, freqs), axis=-1)     # [L, HD]
    cos = np.cos(emb).T.astype(np.float32)            # [HD, L]
    sin = np.sin(emb).T.astype(np.float32)
    sinm = sin.copy()
    sinm[0:64] = -sin[0:64]
    cos_b = cos.astype(bf16)
    sinm_b = sinm.astype(bf16)

    npat = max(len(patterns), 1)
    maskd = np.zeros((npat, 128, 128), dtype=bf16)
    for i, p in enumerate(patterns):
        maskd[i] = p.astype(np.float32).astype(bf16)

    in_maps = []
    for c in range(NCORES):
        b, half = c // 2, c % 2
        rows = slice(half * HPC * HD, (half + 1) * HPC * HD)
        in_maps.append({
            "xT": np.ascontiguousarray(x[b].T).astype(bf16),
            "wqT": np.ascontiguousarray(Wq[rows, :].T).astype(bf16),
            "wkT": np.ascontiguousarray(Wk[rows, :].T).astype(bf16),
            "wvT": np.ascontiguousarray(Wv[rows, :].T).astype(bf16),
            "woT": np.ascontiguousarray(Wo[:, rows].T).astype(bf16),
            "cosd": cos_b,
            "sinmd": sinm_b,
            "maskd": maskd,
        })
    return in_maps


def kernel(x, mask, Wq, Wk, Wv, Wo, _trace=False):
    from concourse.bass_utils import run_bass_kernel_spmd

    x = np.asarray(x, dtype=np.float32)
    mask2d = np.asarray(mask, dtype=np.int32).reshape(L, L)
    key = mask2d.tobytes()
    if key not in _cache:
        kind, patterns, block_pat = _analyze_mask(mask2d)
        nc = _build(kind, block_pat, len(patterns))
        _cache[key] = (nc, patterns)
    nc, patterns = _cache[key]

    in_maps = _prep_inputs(x, mask, np.asarray(Wq, np.float32),
                           np.asarray(Wk, np.float32),
                           np.asarray(Wv, np.float32),
                           np.asarray(Wo, np.float32), patterns)
    res = run_bass_kernel_spmd(nc, in_maps, list(range(NCORES)),
                               trace=_trace)
    y = np.empty((B, L, H), dtype=np.float32)
    for b in range(B):
        acc = res.results[2 * b]["yT"].astype(np.float32) + \
              res.results[2 * b + 1]["yT"].astype(np.float32)
        y[b] = acc.T
    if _trace:
        kernel.last_results = res
    return y


if __name__ == "__main__":
    import reference
    inputs = reference.setup_inputs()
    inputs = {k: np.asarray(v) for k, v in inputs.items()}
    out = kernel(**inputs)
    exp = np.asarray(reference.reference(**{k: v for k, v in inputs.items()}))
    err = np.abs(out - exp).max() / np.abs(exp).max()
    print("rel err (absmax):", err)


# revision 27
# speedup vs baseline: 1.1496x; 1.0020x over previous
"""Trainium2 Bass kernel: causal multi-head attention with RoPE.

Model: B=4, L=2048, H=2048, NH=16 heads, head_dim=128.
  q = x @ Wq.T ; k = x @ Wk.T ; v = x @ Wv.T        (per-head split)
  q, k <- RoPE(q, k)
  attn = softmax(mask(q k^T / sqrt(hd)))
  out  = (attn @ v) heads-concat @ Wo.T

Sharding (8 cores): hybrid batch x tensor-parallel.  Core c handles
batch b = c//2 and heads half*8..half*8+7 with half = c%2.  Wq/Wk/Wv are
column-sharded (8 heads per core), Wo row-sharded; each core produces a
partial y[b] and the host sums the two partials per batch (the unshard
step) and concatenates batches.

Per-core dataflow (all SBUF-resident, bf16 inputs / fp32 accumulation):
  phase A: Q^T, K^T  [128d x 2048pos] per head (d-major) and V
           [128pos x 1024d] pos-major, via PE matmuls; RoPE on Q^T/K^T
           (rotate-half partition shuffle via SBUF->SBUF DMA, the
           elementwise part on DVE).
  phase B: flash-style causal attention per (head, 512-wide q chunk):
           S^T tile = K_blk^T Q_chunk (PE), P = exp(S^T/sqrt(d)) (ACT),
           block-sparse causal structure with a triangular-mask multiply
           on diagonal blocks (DVE), O^T += V_blk P (PE), rowsum via
           ones-matmul (PE), reciprocal+broadcast+scale for the softmax
           normalization (DVE + GPSIMD).
  phase C: y^T partial = Wo_shard O^T (PE) -> DRAM fp32.
"""

import math
import numpy as np

B, L, H, NH, HD = 4, 2048, 2048, 16, 128
ROPE_BASE = 10000.0
NCORES = 8
HPC = 8          # heads per core
QC = 512         # q chunk width
NQC = L // QC    # 4 q chunks
NKB = L // 128   # 16 kp blocks
SCALE = 1.0 / math.sqrt(HD)

_cache = {}


def _analyze_mask(mask2d):
    """Classify each (q_block, kp_block) 128x128 block of the [L, L] mask.

    Returns (block_kind[16][16] with 0=empty,1=full,2=mixed, patterns,
    pattern_idx dict keyed by block coords). mask2d is int32 [L, L],
    rows=q, cols=kp.
    """
    nb = L // 128
    kind = [[0] * nb for _ in range(nb)]
    patterns = []
    pat_key_to_idx = {}
    block_pat = {}
    for qb in range(nb):
        rows = mask2d[qb * 128:(qb + 1) * 128]
        for kb in range(nb):
            blk = rows[:, kb * 128:(kb + 1) * 128]
            s = int(blk.sum())
            if s == 0:
                kind[qb][kb] = 0
            elif s == 128 * 128:
                kind[qb][kb] = 1
            else:
                kind[qb][kb] = 2
                key = blk.tobytes()
                idx = pat_key_to_idx.get(key)
                if idx is None:
                    idx = len(patterns)
                    pat_key_to_idx[key] = idx
                    # stored transposed: S^T tiles are [kp, q]
                    patterns.append(np.ascontiguousarray(blk.T))
                block_pat[(qb, kb)] = idx
    return kind, patterns, block_pat


def _build(kind, block_pat, n_patterns):
    """Build the SPMD bass program (same for all 8 cores)."""
    import concourse.bass as bass
    import concourse.bacc as bacc
    import concourse.mybir as mybir
    import concourse.tile as tile

    fp32 = mybir.dt.float32
    bf16 = mybir.dt.bfloat16
    EXP = mybir.ActivationFunctionType.Exp

    nc = bacc.Bacc("TRN2", target_bir_lowering=False, debug=False)

    xT = nc.dram_tensor("xT", [H, L], bf16, kind="ExternalInput")
    wqT = nc.dram_tensor("wqT", [H, HPC * HD], bf16, kind="ExternalInput")
    wkT = nc.dram_tensor("wkT", [H, HPC * HD], bf16, kind="ExternalInput")
    wvT = nc.dram_tensor("wvT", [H, HPC * HD], bf16, kind="ExternalInput")
    woT = nc.dram_tensor("woT", [HPC * HD, H], bf16, kind="ExternalInput")
    cosd = nc.dram_tensor("cosd", [HD, L], bf16, kind="ExternalInput")
    sinmd = nc.dram_tensor("sinmd", [HD, L], bf16, kind="ExternalInput")
    npat = max(n_patterns, 1)
    maskd = nc.dram_tensor("maskd", [npat, 128, 128], bf16, kind="ExternalInput")
    yT = nc.dram_tensor("yT", [H, L], fp32, kind="ExternalOutput")

    NHC = H // 128  # 16 input-feature blocks

    def qk_phase(tc, w_dram, out_a, wpool, xpool, tpool, pspool, wtag,
                 cos_sb, sinm_sb):
        """Q^T / K^T d-major projection + fused RoPE per (head, chunk)."""
        w_sb = wpool.tile([128, NHC, HPC * HD], bf16, tag="w",
                          name=f"w_{wtag}")
        wr = w_dram[:].rearrange("(a p) m -> p a m", p=128)
        nc.sync.dma_start(out=w_sb[:, 0:4, :], in_=wr[:, 0:4, :])
        nc.sync.dma_start(out=w_sb[:, 4:8, :], in_=wr[:, 4:8, :])
        nc.sync.dma_start(out=w_sb[:, 8:12, :], in_=wr[:, 8:12, :])
        nc.sync.dma_start(out=w_sb[:, 12:16, :], in_=wr[:, 12:16, :])
        for j in range(NQC):
            js = slice(j * QC, (j + 1) * QC)
            x_sb = xpool.tile([128, NHC, QC], bf16, tag="xcols",
                              name=f"x_{wtag}{j}")
            xr = xT[:, js].rearrange("(a p) m -> p a m", p=128)
            nc.sync.dma_start(out=x_sb[:, 0:4, :], in_=xr[:, 0:4, :])
            nc.sync.dma_start(out=x_sb[:, 4:8, :], in_=xr[:, 4:8, :])
            nc.sync.dma_start(out=x_sb[:, 8:12, :], in_=xr[:, 8:12, :])
            nc.sync.dma_start(out=x_sb[:, 12:16, :], in_=xr[:, 12:16, :])
            for h in range(HPC):
                ps = pspool.tile([128, QC], fp32, tag="ps_proj")
                for hc in range(NHC):
                    nc.tensor.matmul(
                        ps[:],
                        w_sb[:, hc, h * HD:(h + 1) * HD],
                        x_sb[:, hc, :],
                        start=(hc == 0), stop=(hc == NHC - 1))
                q = out_a[:, h, js]
                nc.scalar.copy(q, ps[:])
                # rotate-half: pure partition swap, done by SBUF->SBUF DMA
                rq = tpool.tile([128, QC], bf16, tag="rotq")
                nc.sync.dma_start(out=rq[0:64, :], in_=out_a[64:128, h, js])
                nc.sync.dma_start(out=rq[64:128, :], in_=out_a[0:64, h, js])
                nc.vector.tensor_mul(rq[:], rq[:], sinm_sb[:, js])
                nc.vector.tensor_mul(q, q, cos_sb[:, js])
                nc.vector.tensor_add(q, q, rq[:])

    def v_phase(tc, w_dram, va, wpool, xpool, pspool):
        """V pos-major projection (x chunks 256 wide to fit SBUF)."""
        w_sb = wpool.tile([128, NHC, HPC * HD], bf16, tag="w", name="w_v")
        nc.sync.dma_start(
            out=w_sb[:], in_=w_dram[:].rearrange("(a p) m -> p a m", p=128))
        VC = 256
        for j in range(L // VC):
            x_sb = xpool.tile([128, NHC, VC], bf16, tag="xv", name=f"xv{j}")
            nc.sync.dma_start(
                out=x_sb[:],
                in_=xT[:, j * VC:(j + 1) * VC].rearrange(
                    "(a p) m -> p a m", p=128))
            for pb in range(VC // 128):
                psd = [pspool.tile([128, QC], fp32, tag="ps_proj",
                                   name=f"psv{j}_{pb}_{dc}")
                       for dc in range(2)]
                for hc in range(NHC):
                    for dc in range(2):
                        nc.tensor.matmul(
                            psd[dc][:],
                            x_sb[:, hc, pb * 128:(pb + 1) * 128],
                            w_sb[:, hc, dc * QC:(dc + 1) * QC],
                            start=(hc == 0), stop=(hc == NHC - 1))
                for dc in range(2):
                    nc.scalar.copy(
                        va[:, j * (VC // 128) + pb, dc * QC:(dc + 1) * QC],
                        psd[dc][:])

    with tile.TileContext(nc) as tc:
        with tc.tile_pool(name="persist", bufs=1, side="left") as persist:
            # one combined small-constant tile: [trimask patterns | ones]
            cst = persist.tile([128, npat * 128 + 128], bf16, tag="cst")
            for p in range(n_patterns):
                nc.sync.dma_start(out=cst[:, p * 128:(p + 1) * 128],
                                  in_=maskd[p])
            ones_col = npat * 128
            nc.vector.memset(cst[:, ones_col:ones_col + 128], 1.0)
            onesf = persist.tile([128, 128], fp32, tag="onesf")
            nc.vector.memset(onesf[:], 1.0)
            QTa = persist.tile([HD, HPC, L], bf16, tag="qta")
            KTa = persist.tile([HD, HPC, L], bf16, tag="kta")

            # ---------------- phase A: projections + RoPE ----------------
            # Manual pool lifetimes (non-LIFO): weights/x/rope tables are
            # freed before attention while Va spans V-phase..attention.
            wpool_cm = tc.tile_pool(name="wpool", bufs=2, side="right")
            wpool = wpool_cm.__enter__()
            ropec_cm = tc.tile_pool(name="ropec", bufs=1, side="right")
            ropec = ropec_cm.__enter__()
            psp_cm = tc.tile_pool(name="ps_proj", bufs=4, space="PSUM")
            psp = psp_cm.__enter__()

            cos_sb = ropec.tile([HD, L], bf16, tag="cos")
            sinm_sb = ropec.tile([HD, L], bf16, tag="sinm")
            nc.sync.dma_start(out=cos_sb[:], in_=cosd[:])
            nc.sync.dma_start(out=sinm_sb[:], in_=sinmd[:])

            xqk_cm = tc.tile_pool(name="xqk", bufs=2, side="right")
            xqk = xqk_cm.__enter__()
            tpool_cm = tc.tile_pool(name="tpool", bufs=3, side="right")
            tpool = tpool_cm.__enter__()
            qk_phase(tc, wqT, QTa, wpool, xqk, tpool, psp, "q",
                     cos_sb, sinm_sb)
            qk_phase(tc, wkT, KTa, wpool, xqk, tpool, psp, "k",
                     cos_sb, sinm_sb)
            tpool_cm.__exit__(None, None, None)
            xqk_cm.__exit__(None, None, None)
            ropec_cm.__exit__(None, None, None)

            vp_cm = tc.tile_pool(name="vp", bufs=1, side="left")
            vp_outer = vp_cm.__enter__()
            Va = vp_outer.tile([128, NKB, HPC * HD], bf16, tag="va")
            xv_cm = tc.tile_pool(name="xv", bufs=2, side="right")
            xv = xv_cm.__enter__()
            v_phase(tc, wvT, Va, wpool, xv, psp)
            xv_cm.__exit__(None, None, None)
            wpool_cm.__exit__(None, None, None)
            psp_cm.__exit__(None, None, None)

            # -------- phase B + C under Va's lifetime --------
            _attn_and_out(tc, nc, kind, block_pat, QTa, KTa, Va,
                          cst, ones_col, onesf, woT, yT, fp32, bf16, EXP)
            vp_cm.__exit__(None, None, None)

    nc.compile()
    return nc


def _attn_and_out(tc, nc, kind, block_pat, QTa, KTa, Va, cst, ones_col,
                  onesf, woT, yT, fp32, bf16, EXP):
    ones_sb = cst[:, ones_col:ones_col + 1]
    with tc.tile_pool(name="otp", bufs=1, side="left") as otp, \
         tc.tile_pool(name="wo", bufs=1, side="left") as wop:
        OTa = otp.tile([HD, HPC, L], bf16, tag="ota")
        wo_sb = wop.tile([128, HPC, H], bf16, tag="wo")
        # prefetch Wo during attention
        nc.sync.dma_start(
            out=wo_sb[:], in_=woT[:].rearrange("(a p) m -> p a m", p=128))

        # ---------------- phase B: attention ----------------
        # per-(head, q-chunk) flash loop; softmax normalization broadcast
        # via K=1 PE outer product + ACT copy (keeps gpsimd free)
        with tc.tile_pool(name="pp", bufs=4, side="right") as ppool, \
             tc.tile_pool(name="rr", bufs=2, side="right") as rpool, \
             tc.tile_pool(name="bb", bufs=3, side="right") as bpool, \
             tc.tile_pool(name="ps_s", bufs=3, space="PSUM") as ps_s, \
             tc.tile_pool(name="ps_o", bufs=2, space="PSUM") as ps_o, \
             tc.tile_pool(name="ps_r", bufs=2, space="PSUM") as ps_r:
            for h in range(HPC):
                for j in range(NQC):
                    blocks = []
                    for i in range(NKB):
                        live = [t for t in range(4)
                                if kind[4 * j + t][i] != 0]
                        if live:
                            blocks.append((i, live))
                    if not blocks:
                        continue
                    pso = ps_o.tile([128, QC], fp32, tag="pso",
                                    name=f"pso{h}_{j}")
                    psr = ps_r.tile([128, QC], fp32, tag="psr",
                                    name=f"psr{h}_{j}")
                    last = len(blocks) - 1

                    def emit_s(bi):
                        i, live = blocks[bi]
                        t0, t1 = live[0], live[-1]
                        w0, w1 = t0 * 128, (t1 + 1) * 128
                        pss = ps_s.tile([128, QC], fp32, tag="pss",
                                        name=f"pss{h}_{j}_{i}")
                        nc.tensor.matmul(
                            pss[:, w0:w1],
                            KTa[:, h, i * 128:(i + 1) * 128],
                            QTa[:, h, j * QC + w0:j * QC + w1],
                            start=True, stop=True)
                        P = ppool.tile([128, QC], bf16, tag="p",
                                       name=f"p{h}_{j}_{i}")
                        first = (bi == 0)
                        if w0 > 0 and first:
                            nc.vector.memset(P[:, 0:w0], 0.0)
                        if w1 < QC and first:
                            nc.vector.memset(P[:, w1:QC], 0.0)
                        nc.scalar.activation(P[:, w0:w1], pss[:, w0:w1],
                                             EXP, scale=SCALE)
                        for t in range(t0, t1 + 1):
                            qb = 4 * j + t
                            if kind[qb][i] == 0:
                                nc.vector.memset(
                                    P[:, t * 128:(t + 1) * 128], 0.0)
                            elif kind[qb][i] == 2:
                                pat = block_pat[(qb, i)]
                                nc.vector.tensor_mul(
                                    P[:, t * 128:(t + 1) * 128],
                                    P[:, t * 128:(t + 1) * 128],
                                    cst[:, pat * 128:(pat + 1) * 128])
                        return P, w0, first

                    def emit_ov(bi, P, w0, first):
                        i, live = blocks[bi]
                        m0 = 0 if first else w0
                        nc.tensor.matmul(
                            pso[:, m0:QC],
                            Va[:, i, h * HD:(h + 1) * HD],
                            P[:, m0:QC],
                            start=first, stop=(bi == last))
                        nc.tensor.matmul(
                            psr[0:1, m0:QC], ones_sb, P[:, m0:QC],
                            start=first, stop=(bi == last))

                    # 2-deep software pipeline: keep two S tiles in flight
                    # ahead of their O/rowsum consumers so the PE never
                    # waits on the ACT exp latency
                    LOOK = 2
                    pend = []
                    for bi in range(len(blocks)):
                        pend.append((bi,) + emit_s(bi))
                        if len(pend) > LOOK:
                            b0, P0, w00, f0 = pend.pop(0)
                            emit_ov(b0, P0, w00, f0)
                    for b0, P0, w00, f0 in pend:
                        emit_ov(b0, P0, w00, f0)
                    r_sb = rpool.tile([128, QC], fp32, tag="r",
                                      name=f"r{h}_{j}")
                    nc.vector.reciprocal_approx_fast(
                        out=r_sb[0:1, :], in_=psr[0:1, :])
                    rb_sb = rpool.tile([128, QC], bf16, tag="rb",
                                       name=f"rb{h}_{j}")
                    nc.vector.tensor_copy(rb_sb[0:1, :], r_sb[0:1, :])
                    bc_sb = bpool.tile([128, QC], bf16, tag="bc",
                                       name=f"bc{h}_{j}")
                    nc.gpsimd.partition_broadcast(bc_sb[:], rb_sb[0:1, :])
                    nc.vector.tensor_mul(
                        OTa[:, h, j * QC:(j + 1) * QC], pso[:], bc_sb[:])

        # ---------------- phase C: output projection ----------------
        with tc.tile_pool(name="ysb", bufs=3, side="right") as ypool, \
             tc.tile_pool(name="ps_c", bufs=4, space="PSUM") as ps_c:
            for oc in range(H // 128):
                for j in range(NQC):
                    ps = ps_c.tile([128, QC], fp32, tag="psc")
                    for fc in range(HPC):
                        nc.tensor.matmul(
                            ps[:],
                            wo_sb[:, fc, oc * 128:(oc + 1) * 128],
                            OTa[:, fc, j * QC:(j + 1) * QC],
                            start=(fc == 0), stop=(fc == HPC - 1))
                    y_sb = ypool.tile([128, QC], fp32, tag="y")
                    nc.vector.tensor_copy(y_sb[:], ps[:])
                    nc.sync.dma_start(
                        out=yT[oc * 128:(oc + 1) * 128,
                               j * QC:(j + 1) * QC],
                        in_=y_sb[:])


def _prep_inputs(x, mask, Wq, Wk, Wv, Wo, patterns):
    import ml_dtypes
    bf16 = ml_dtypes.bfloat16

    # RoPE tables, d-major [HD, L]
    inv_freq = 1.0 / (ROPE_BASE ** (np.arange(0, HD, 2, dtype=np.float64)
                                    / HD))
    t = np.arange(L, dtype=np.float64)
    freqs = np.outer(t, inv_freq)                     # [L, HD/2]
    emb = np.concatenate((freqs, freqs), axis=-1)     # [L, HD]
    cos = np.cos(emb).T.astype(np.float32)            # [HD, L]
    sin = np.sin(emb).T.astype(np.float32)
    sinm = sin.copy()
    sinm[0:64] = -sin[0:64]
    cos_b = cos.astype(bf16)
    sinm_b = sinm.astype(bf16)

    npat = max(len(patterns), 1)
    maskd = np.zeros((npat, 128, 128), dtype=bf16)
    for i, p in enumerate(patterns):
        maskd[i] = p.astype(np.float32).astype(bf16)

    in_maps = []
    for c in range(NCORES):
        b, half = c // 2, c % 2
        rows = slice(half * HPC * HD, (half + 1) * HPC * HD)
        in_maps.append({
            "xT": np.ascontiguousarray(x[b].T).astype(bf16),
            "wqT": np.ascontiguousarray(Wq[rows, :].T).astype(bf16),
            "wkT": np.ascontiguousarray(Wk[rows, :].T).astype(bf16),
            "wvT": np.ascontiguousarray(Wv[rows, :].T).astype(bf16),
            "woT": np.ascontiguousarray(Wo[:, rows].T).astype(bf16),
            "cosd": cos_b,
            "sinmd": sinm_b,
            "maskd": maskd,
        })
    return in_maps


def kernel(x, mask, Wq, Wk, Wv, Wo, _trace=False):
    from concourse.bass_utils import run_bass_kernel_spmd

    x = np.asarray(x, dtype=np.float32)
    mask2d = np.asarray(mask, dtype=np.int32).reshape(L, L)
    key = mask2d.tobytes()
    if key not in _cache:
        kind, patterns, block_pat = _analyze_mask(mask2d)
        nc = _build(kind, block_pat, len(patterns))
        _cache[key] = (nc, patterns)
    nc, patterns = _cache[key]

    in_maps = _prep_inputs(x, mask, np.asarray(Wq, np.float32),
                           np.asarray(Wk, np.float32),
                           np.asarray(Wv, np.float32),
                           np.asarray(Wo, np.float32), patterns)
    res = run_bass_kernel_spmd(nc, in_maps, list(range(NCORES)),
                               trace=_trace)
    y = np.empty((B, L, H), dtype=np.float32)
    for b in range(B):
        acc = res.results[2 * b]["yT"].astype(np.float32) + \
              res.results[2 * b + 1]["yT"].astype(np.float32)
        y[b] = acc.T
    if _trace:
        kernel.last_results = res
    return y


if __name__ == "__main__":
    import reference
    inputs = reference.setup_inputs()
    inputs = {k: np.asarray(v) for k, v in inputs.items()}
    out = kernel(**inputs)
    exp = np.asarray(reference.reference(**{k: v for k, v in inputs.items()}))
    err = np.abs(out - exp).max() / np.abs(exp).max()
    print("rel err (absmax):", err)


# revision 29
# speedup vs baseline: 1.1670x; 1.0152x over previous
"""Trainium2 Bass kernel: causal multi-head attention with RoPE.

Model: B=4, L=2048, H=2048, NH=16 heads, head_dim=128.
  q = x @ Wq.T ; k = x @ Wk.T ; v = x @ Wv.T        (per-head split)
  q, k <- RoPE(q, k)
  attn = softmax(mask(q k^T / sqrt(hd)))
  out  = (attn @ v) heads-concat @ Wo.T

Sharding (8 cores): hybrid batch x tensor-parallel.  Core c handles
batch b = c//2 and heads half*8..half*8+7 with half = c%2.  Wq/Wk/Wv are
column-sharded (8 heads per core), Wo row-sharded; each core produces a
partial y[b] and the host sums the two partials per batch (the unshard
step) and concatenates batches.

Per-core dataflow (all SBUF-resident, bf16 inputs / fp32 accumulation):
  phase A: Q^T, K^T  [128d x 2048pos] per head (d-major) and V
           [128pos x 1024d] pos-major, via PE matmuls; RoPE on Q^T/K^T
           (rotate-half partition shuffle via SBUF->SBUF DMA, the
           elementwise part on DVE).
  phase B: flash-style causal attention per (head, 512-wide q chunk):
           S^T tile = K_blk^T Q_chunk (PE), P = exp(S^T/sqrt(d)) (ACT),
           block-sparse causal structure with a triangular-mask multiply
           on diagonal blocks (DVE), O^T += V_blk P (PE), rowsum via
           ones-matmul (PE), reciprocal+broadcast+scale for the softmax
           normalization (DVE + GPSIMD).
  phase C: y^T partial = Wo_shard O^T (PE) -> DRAM fp32.
"""

import math
import numpy as np

B, L, H, NH, HD = 4, 2048, 2048, 16, 128
ROPE_BASE = 10000.0
NCORES = 8
HPC = 8          # heads per core
QC = 512         # q chunk width
NQC = L // QC    # 4 q chunks
NKB = L // 128   # 16 kp blocks
SCALE = 1.0 / math.sqrt(HD)

_cache = {}


def _analyze_mask(mask2d):
    """Classify each (q_block, kp_block) 128x128 block of the [L, L] mask.

    Returns (block_kind[16][16] with 0=empty,1=full,2=mixed, patterns,
    pattern_idx dict keyed by block coords). mask2d is int32 [L, L],
    rows=q, cols=kp.
    """
    nb = L // 128
    kind = [[0] * nb for _ in range(nb)]
    patterns = []
    pat_key_to_idx = {}
    block_pat = {}
    for qb in range(nb):
        rows = mask2d[qb * 128:(qb + 1) * 128]
        for kb in range(nb):
            blk = rows[:, kb * 128:(kb + 1) * 128]
            s = int(blk.sum())
            if s == 0:
                kind[qb][kb] = 0
            elif s == 128 * 128:
                kind[qb][kb] = 1
            else:
                kind[qb][kb] = 2
                key = blk.tobytes()
                idx = pat_key_to_idx.get(key)
                if idx is None:
                    idx = len(patterns)
                    pat_key_to_idx[key] = idx
                    # stored transposed: S^T tiles are [kp, q]
                    patterns.append(np.ascontiguousarray(blk.T))
                block_pat[(qb, kb)] = idx
    return kind, patterns, block_pat


def _build(kind, block_pat, n_patterns):
    """Build the SPMD bass program (same for all 8 cores)."""
    import concourse.bass as bass
    import concourse.bacc as bacc
    import concourse.mybir as mybir
    import concourse.tile as tile

    fp32 = mybir.dt.float32
    bf16 = mybir.dt.bfloat16
    EXP = mybir.ActivationFunctionType.Exp

    nc = bacc.Bacc("TRN2", target_bir_lowering=False, debug=False)

    xT = nc.dram_tensor("xT", [H, L], bf16, kind="ExternalInput")
    wqT = nc.dram_tensor("wqT", [H, HPC * HD], bf16, kind="ExternalInput")
    wkT = nc.dram_tensor("wkT", [H, HPC * HD], bf16, kind="ExternalInput")
    wvT = nc.dram_tensor("wvT", [H, HPC * HD], bf16, kind="ExternalInput")
    woT = nc.dram_tensor("woT", [HPC * HD, H], bf16, kind="ExternalInput")
    cosd = nc.dram_tensor("cosd", [HD, L], bf16, kind="ExternalInput")
    sinmd = nc.dram_tensor("sinmd", [HD, L], bf16, kind="ExternalInput")
    npat = max(n_patterns, 1)
    maskd = nc.dram_tensor("maskd", [npat, 128, 128], bf16, kind="ExternalInput")
    yT = nc.dram_tensor("yT", [H, L], fp32, kind="ExternalOutput")

    NHC = H // 128  # 16 input-feature blocks

    def qk_phase(tc, w_dram, out_a, wpool, xpool, tpool, pspool, wtag,
                 cos_sb, sinm_sb):
        """Q^T / K^T d-major projection + fused RoPE per (head, chunk)."""
        w_sb = wpool.tile([128, NHC, HPC * HD], bf16, tag="w",
                          name=f"w_{wtag}")
        wr = w_dram[:].rearrange("(a p) m -> p a m", p=128)
        nc.sync.dma_start(out=w_sb[:, 0:4, :], in_=wr[:, 0:4, :])
        nc.sync.dma_start(out=w_sb[:, 4:8, :], in_=wr[:, 4:8, :])
        nc.sync.dma_start(out=w_sb[:, 8:12, :], in_=wr[:, 8:12, :])
        nc.sync.dma_start(out=w_sb[:, 12:16, :], in_=wr[:, 12:16, :])
        for j in range(NQC):
            js = slice(j * QC, (j + 1) * QC)
            x_sb = xpool.tile([128, NHC, QC], bf16, tag="xcols",
                              name=f"x_{wtag}{j}")
            xr = xT[:, js].rearrange("(a p) m -> p a m", p=128)
            nc.sync.dma_start(out=x_sb[:, 0:4, :], in_=xr[:, 0:4, :])
            nc.sync.dma_start(out=x_sb[:, 4:8, :], in_=xr[:, 4:8, :])
            nc.sync.dma_start(out=x_sb[:, 8:12, :], in_=xr[:, 8:12, :])
            nc.sync.dma_start(out=x_sb[:, 12:16, :], in_=xr[:, 12:16, :])
            for h in range(HPC):
                ps = pspool.tile([128, QC], fp32, tag="ps_proj")
                for hc in range(NHC):
                    nc.tensor.matmul(
                        ps[:],
                        w_sb[:, hc, h * HD:(h + 1) * HD],
                        x_sb[:, hc, :],
                        start=(hc == 0), stop=(hc == NHC - 1))
                q = out_a[:, h, js]
                nc.scalar.copy(q, ps[:])
                # rotate-half: pure partition swap, done by SBUF->SBUF DMA
                rq = tpool.tile([128, QC], bf16, tag="rotq")
                nc.sync.dma_start(out=rq[0:64, :], in_=out_a[64:128, h, js])
                nc.sync.dma_start(out=rq[64:128, :], in_=out_a[0:64, h, js])
                nc.vector.tensor_mul(rq[:], rq[:], sinm_sb[:, js])
                nc.vector.tensor_mul(q, q, cos_sb[:, js])
                nc.vector.tensor_add(q, q, rq[:])

    def v_phase(tc, w_dram, va, wpool, xpool, pspool):
        """V pos-major projection (x chunks 256 wide to fit SBUF)."""
        w_sb = wpool.tile([128, NHC, HPC * HD], bf16, tag="w", name="w_v")
        nc.sync.dma_start(
            out=w_sb[:], in_=w_dram[:].rearrange("(a p) m -> p a m", p=128))
        VC = 256
        for j in range(L // VC):
            x_sb = xpool.tile([128, NHC, VC], bf16, tag="xv", name=f"xv{j}")
            nc.sync.dma_start(
                out=x_sb[:],
                in_=xT[:, j * VC:(j + 1) * VC].rearrange(
                    "(a p) m -> p a m", p=128))
            for pb in range(VC // 128):
                psd = [pspool.tile([128, QC], fp32, tag="ps_proj",
                                   name=f"psv{j}_{pb}_{dc}")
                       for dc in range(2)]
                for hc in range(NHC):
                    for dc in range(2):
                        nc.tensor.matmul(
                            psd[dc][:],
                            x_sb[:, hc, pb * 128:(pb + 1) * 128],
                            w_sb[:, hc, dc * QC:(dc + 1) * QC],
                            start=(hc == 0), stop=(hc == NHC - 1))
                for dc in range(2):
                    nc.scalar.copy(
                        va[:, j * (VC // 128) + pb, dc * QC:(dc + 1) * QC],
                        psd[dc][:])

    with tile.TileContext(nc) as tc:
        with tc.tile_pool(name="persist", bufs=1, side="left") as persist:
            # one combined small-constant tile: [trimask patterns | ones]
            cst = persist.tile([128, npat * 128 + 128], bf16, tag="cst")
            for p in range(n_patterns):
                nc.gpsimd.dma_start(out=cst[:, p * 128:(p + 1) * 128],
                                    in_=maskd[p])
            ones_col = npat * 128
            nc.vector.memset(cst[:, ones_col:ones_col + 128], 1.0)
            onesf = persist.tile([128, 128], fp32, tag="onesf")
            nc.vector.memset(onesf[:], 1.0)
            QTa = persist.tile([HD, HPC, L], bf16, tag="qta")
            KTa = persist.tile([HD, HPC, L], bf16, tag="kta")

            # ---------------- phase A: projections + RoPE ----------------
            # Manual pool lifetimes (non-LIFO): weights/x/rope tables are
            # freed before attention while Va spans V-phase..attention.
            wpool_cm = tc.tile_pool(name="wpool", bufs=2, side="right")
            wpool = wpool_cm.__enter__()
            ropec_cm = tc.tile_pool(name="ropec", bufs=1, side="right")
            ropec = ropec_cm.__enter__()
            psp_cm = tc.tile_pool(name="ps_proj", bufs=4, space="PSUM")
            psp = psp_cm.__enter__()

            cos_sb = ropec.tile([HD, L], bf16, tag="cos")
            sinm_sb = ropec.tile([HD, L], bf16, tag="sinm")
            nc.gpsimd.dma_start(out=cos_sb[:], in_=cosd[:])
            nc.gpsimd.dma_start(out=sinm_sb[:], in_=sinmd[:])

            xv_cm = tc.tile_pool(name="xv", bufs=2, side="right")
            xv = xv_cm.__enter__()
            xqk_cm = tc.tile_pool(name="xqk", bufs=2, side="right")
            xqk = xqk_cm.__enter__()
            tpool_cm = tc.tile_pool(name="tpool", bufs=2, side="right")
            tpool = tpool_cm.__enter__()
            qk_phase(tc, wqT, QTa, wpool, xqk, tpool, psp, "q",
                     cos_sb, sinm_sb)
            qk_phase(tc, wkT, KTa, wpool, xqk, tpool, psp, "k",
                     cos_sb, sinm_sb)
            tpool_cm.__exit__(None, None, None)
            xqk_cm.__exit__(None, None, None)
            vp_cm = tc.tile_pool(name="vp", bufs=1, side="left")
            vp_outer = vp_cm.__enter__()
            Va = vp_outer.tile([128, NKB, HPC * HD], bf16, tag="va")
            v_phase(tc, wvT, Va, wpool, xv, psp)
            xv_cm.__exit__(None, None, None)
            ropec_cm.__exit__(None, None, None)
            wpool_cm.__exit__(None, None, None)
            psp_cm.__exit__(None, None, None)

            # -------- phase B + C under Va's lifetime --------
            _attn_and_out(tc, nc, kind, block_pat, QTa, KTa, Va,
                          cst, ones_col, onesf, woT, yT, fp32, bf16, EXP)
            vp_cm.__exit__(None, None, None)

    nc.compile()
    return nc


def _attn_and_out(tc, nc, kind, block_pat, QTa, KTa, Va, cst, ones_col,
                  onesf, woT, yT, fp32, bf16, EXP):
    ones_sb = cst[:, ones_col:ones_col + 1]
    with tc.tile_pool(name="otp", bufs=1, side="left") as otp, \
         tc.tile_pool(name="wo", bufs=1, side="left") as wop:
        OTa = otp.tile([HD, HPC, L], bf16, tag="ota")
        wo_sb = wop.tile([128, HPC, H], bf16, tag="wo")
        # prefetch Wo during attention
        nc.sync.dma_start(
            out=wo_sb[:], in_=woT[:].rearrange("(a p) m -> p a m", p=128))

        # ---------------- phase B: attention ----------------
        # per-(head, q-chunk) flash loop; softmax normalization broadcast
        # via K=1 PE outer product + ACT copy (keeps gpsimd free)
        with tc.tile_pool(name="pp", bufs=4, side="right") as ppool, \
             tc.tile_pool(name="rr", bufs=2, side="right") as rpool, \
             tc.tile_pool(name="bb", bufs=3, side="right") as bpool, \
             tc.tile_pool(name="ps_s", bufs=3, space="PSUM") as ps_s, \
             tc.tile_pool(name="ps_o", bufs=2, space="PSUM") as ps_o, \
             tc.tile_pool(name="ps_r", bufs=2, space="PSUM") as ps_r:
            for h in range(HPC):
                for j in range(NQC):
                    blocks = []
                    for i in range(NKB):
                        live = [t for t in range(4)
                                if kind[4 * j + t][i] != 0]
                        if live:
                            blocks.append((i, live))
                    if not blocks:
                        continue
                    pso = ps_o.tile([128, QC], fp32, tag="pso",
                                    name=f"pso{h}_{j}")
                    psr = ps_r.tile([128, QC], fp32, tag="psr",
                                    name=f"psr{h}_{j}")
                    last = len(blocks) - 1

                    def emit_s(bi):
                        i, live = blocks[bi]
                        t0, t1 = live[0], live[-1]
                        w0, w1 = t0 * 128, (t1 + 1) * 128
                        pss = ps_s.tile([128, QC], fp32, tag="pss",
                                        name=f"pss{h}_{j}_{i}")
                        nc.tensor.matmul(
                            pss[:, w0:w1],
                            KTa[:, h, i * 128:(i + 1) * 128],
                            QTa[:, h, j * QC + w0:j * QC + w1],
                            start=True, stop=True)
                        P = ppool.tile([128, QC], bf16, tag="p",
                                       name=f"p{h}_{j}_{i}")
                        first = (bi == 0)
                        if w0 > 0 and first:
                            nc.vector.memset(P[:, 0:w0], 0.0)
                        if w1 < QC and first:
                            nc.vector.memset(P[:, w1:QC], 0.0)
                        nc.scalar.activation(P[:, w0:w1], pss[:, w0:w1],
                                             EXP, scale=SCALE)
                        for t in range(t0, t1 + 1):
                            qb = 4 * j + t
                            if kind[qb][i] == 0:
                                nc.vector.memset(
                                    P[:, t * 128:(t + 1) * 128], 0.0)
                            elif kind[qb][i] == 2:
                                pat = block_pat[(qb, i)]
                                nc.vector.tensor_mul(
                                    P[:, t * 128:(t + 1) * 128],
                                    P[:, t * 128:(t + 1) * 128],
                                    cst[:, pat * 128:(pat + 1) * 128])
                        return P, w0, first

                    def emit_ov(bi, P, w0, first):
                        i, live = blocks[bi]
                        m0 = 0 if first else w0
                        nc.tensor.matmul(
                            pso[:, m0:QC],
                            Va[:, i, h * HD:(h + 1) * HD],
                            P[:, m0:QC],
                            start=first, stop=(bi == last))
                        nc.tensor.matmul(
                            psr[0:1, m0:QC], ones_sb, P[:, m0:QC],
                            start=first, stop=(bi == last))

                    # 2-deep software pipeline: keep two S tiles in flight
                    # ahead of their O/rowsum consumers so the PE never
                    # waits on the ACT exp latency
                    LOOK = 2
                    pend = []
                    for bi in range(len(blocks)):
                        pend.append((bi,) + emit_s(bi))
                        if len(pend) > LOOK:
                            b0, P0, w00, f0 = pend.pop(0)
                            emit_ov(b0, P0, w00, f0)
                    for b0, P0, w00, f0 in pend:
                        emit_ov(b0, P0, w00, f0)
                    r_sb = rpool.tile([128, QC], fp32, tag="r",
                                      name=f"r{h}_{j}")
                    nc.vector.reciprocal_approx_fast(
                        out=r_sb[0:1, :], in_=psr[0:1, :])
                    rb_sb = rpool.tile([128, QC], bf16, tag="rb",
                                       name=f"rb{h}_{j}")
                    nc.vector.tensor_copy(rb_sb[0:1, :], r_sb[0:1, :])
                    bc_sb = bpool.tile([128, QC], bf16, tag="bc",
                                       name=f"bc{h}_{j}")
                    nc.gpsimd.partition_broadcast(bc_sb[:], rb_sb[0:1, :])
                    nc.vector.tensor_mul(
                        OTa[:, h, j * QC:(j + 1) * QC], pso[:], bc_sb[:])

        # ---------------- phase C: output projection ----------------
        with tc.tile_pool(name="ysb", bufs=3, side="right") as ypool, \
             tc.tile_pool(name="ps_c", bufs=4, space="PSUM") as ps_c:
            for oc in range(H // 128):
                for j in range(NQC):
                    ps = ps_c.tile([128, QC], fp32, tag="psc")
                    for fc in range(HPC):
                        nc.tensor.matmul(
                            ps[:],
                            wo_sb[:, fc, oc * 128:(oc + 1) * 128],
                            OTa[:, fc, j * QC:(j + 1) * QC],
                            start=(fc == 0), stop=(fc == HPC - 1))
                    y_sb = ypool.tile([128, QC], fp32, tag="y")
                    nc.vector.tensor_copy(y_sb[:], ps[:])
                    nc.sync.dma_start(
                        out=yT[oc * 128:(oc + 1) * 128,
                               j * QC:(j + 1) * QC],
                        in_=y_sb[:])


def _prep_inputs(x, mask, Wq, Wk, Wv, Wo, patterns):
    import ml_dtypes
    bf16 = ml_dtypes.bfloat16

    # RoPE tables, d-major [HD, L]
    inv_freq = 1.0 / (ROPE_BASE ** (np.arange(0, HD, 2, dtype=np.float64)
                                    / HD))
    t = np.arange(L, dtype=np.float64)
    freqs = np.outer(t, inv_freq)                     # [L, HD/2]
    emb = np.concatenate((freqs, freqs), axis=-1)     # [L, HD]
    cos = np.cos(emb).T.astype(np.float32)            # [HD, L]
    sin = np.sin(emb).T.astype(np.float32)
    sinm = sin.copy()
    sinm[0:64] = -sin[0:64]
    cos_b = cos.astype(bf16)
    sinm_b = sinm.astype(bf16)

    npat = max(len(patterns), 1)
    maskd = np.zeros((npat, 128, 128), dtype=bf16)
    for i, p in enumerate(patterns):
        maskd[i] = p.astype(np.float32).astype(bf16)

    in_maps = []
    for c in range(NCORES):
        b, half = c // 2, c % 2
        rows = slice(half * HPC * HD, (half + 1) * HPC * HD)
        in_maps.append({
            "xT": np.ascontiguousarray(x[b].T).astype(bf16),
            "wqT": np.ascontiguousarray(Wq[rows, :].T).astype(bf16),
            "wkT": np.ascontiguousarray(Wk[rows, :].T).astype(bf16),
            "wvT": np.ascontiguousarray(Wv[rows, :].T).astype(bf16),
            "woT": np.ascontiguousarray(Wo[:, rows].T).astype(bf16),
            "cosd": cos_b,
            "sinmd": sinm_b,
            "maskd": maskd,
        })
    return in_maps


def kernel(x, mask, Wq, Wk, Wv, Wo, _trace=False):
    from concourse.bass_utils import run_bass_kernel_spmd

    x = np.asarray(x, dtype=np.float32)
    mask2d = np.asarray(mask, dtype=np.int32).reshape(L, L)
    key = mask2d.tobytes()
    if key not in _cache:
        kind, patterns, block_pat = _analyze_mask(mask2d)
        nc = _build(kind, block_pat, len(patterns))
        _cache[key] = (nc, patterns)
    nc, patterns = _cache[key]

    in_maps = _prep_inputs(x, mask, np.asarray(Wq, np.float32),
                           np.asarray(Wk, np.float32),
                           np.asarray(Wv, np.float32),
                           np.asarray(Wo, np.float32), patterns)
    res = run_bass_kernel_spmd(nc, in_maps, list(range(NCORES)),
                               trace=_trace)
    y = np.empty((B, L, H), dtype=np.float32)
    for b in range(B):
        acc = res.results[2 * b]["yT"].astype(np.float32) + \
              res.results[2 * b + 1]["yT"].astype(np.float32)
        y[b] = acc.T
    if _trace:
        kernel.last_results = res
    return y


if __name__ == "__main__":
    import reference
    inputs = reference.setup_inputs()
    inputs = {k: np.asarray(v) for k, v in inputs.items()}
    out = kernel(**inputs)
    exp = np.asarray(reference.reference(**{k: v for k, v in inputs.items()}))
    err = np.abs(out - exp).max() / np.abs(exp).max()
    print("rel err (absmax):", err)


# revision 30
# speedup vs baseline: 1.1695x; 1.0021x over previous
"""Trainium2 Bass kernel: causal multi-head attention with RoPE.

Model: B=4, L=2048, H=2048, NH=16 heads, head_dim=128.
  q = x @ Wq.T ; k = x @ Wk.T ; v = x @ Wv.T        (per-head split)
  q, k <- RoPE(q, k)
  attn = softmax(mask(q k^T / sqrt(hd)))
  out  = (attn @ v) heads-concat @ Wo.T

Sharding (8 cores): hybrid batch x tensor-parallel.  Core c handles
batch b = c//2 and heads half*8..half*8+7 with half = c%2.  Wq/Wk/Wv are
column-sharded (8 heads per core), Wo row-sharded; each core produces a
partial y[b] and the host sums the two partials per batch (the unshard
step) and concatenates batches.

Per-core dataflow (all SBUF-resident, bf16 inputs / fp32 accumulation):
  phase A: Q^T, K^T  [128d x 2048pos] per head (d-major) and V
           [128pos x 1024d] pos-major, via PE matmuls; RoPE on Q^T/K^T
           (rotate-half partition shuffle via SBUF->SBUF DMA, the
           elementwise part on DVE).
  phase B: flash-style causal attention per (head, 512-wide q chunk):
           S^T tile = K_blk^T Q_chunk (PE), P = exp(S^T/sqrt(d)) (ACT),
           block-sparse causal structure with a triangular-mask multiply
           on diagonal blocks (DVE), O^T += V_blk P (PE), rowsum via
           ones-matmul (PE), reciprocal+broadcast+scale for the softmax
           normalization (DVE + GPSIMD).
  phase C: y^T partial = Wo_shard O^T (PE) -> DRAM fp32.
"""

import math
import numpy as np

B, L, H, NH, HD = 4, 2048, 2048, 16, 128
ROPE_BASE = 10000.0
NCORES = 8
HPC = 8          # heads per core
QC = 512         # q chunk width
NQC = L // QC    # 4 q chunks
NKB = L // 128   # 16 kp blocks
SCALE = 1.0 / math.sqrt(HD)

_cache = {}


def _analyze_mask(mask2d):
    """Classify each (q_block, kp_block) 128x128 block of the [L, L] mask.

    Returns (block_kind[16][16] with 0=empty,1=full,2=mixed, patterns,
    pattern_idx dict keyed by block coords). mask2d is int32 [L, L],
    rows=q, cols=kp.
    """
    nb = L // 128
    kind = [[0] * nb for _ in range(nb)]
    patterns = []
    pat_key_to_idx = {}
    block_pat = {}
    for qb in range(nb):
        rows = mask2d[qb * 128:(qb + 1) * 128]
        for kb in range(nb):
            blk = rows[:, kb * 128:(kb + 1) * 128]
            s = int(blk.sum())
            if s == 0:
                kind[qb][kb] = 0
            elif s == 128 * 128:
                kind[qb][kb] = 1
            else:
                kind[qb][kb] = 2
                key = blk.tobytes()
                idx = pat_key_to_idx.get(key)
                if idx is None:
                    idx = len(patterns)
                    pat_key_to_idx[key] = idx
                    # stored transposed: S^T tiles are [kp, q]
                    patterns.append(np.ascontiguousarray(blk.T))
                block_pat[(qb, kb)] = idx
    return kind, patterns, block_pat


def _build(kind, block_pat, n_patterns):
    """Build the SPMD bass program (same for all 8 cores)."""
    import concourse.bass as bass
    import concourse.bacc as bacc
    import concourse.mybir as mybir
    import concourse.tile as tile

    fp32 = mybir.dt.float32
    bf16 = mybir.dt.bfloat16
    EXP = mybir.ActivationFunctionType.Exp

    nc = bacc.Bacc("TRN2", target_bir_lowering=False, debug=False)

    xT = nc.dram_tensor("xT", [H, L], bf16, kind="ExternalInput")
    wqT = nc.dram_tensor("wqT", [H, HPC * HD], bf16, kind="ExternalInput")
    wkT = nc.dram_tensor("wkT", [H, HPC * HD], bf16, kind="ExternalInput")
    wvT = nc.dram_tensor("wvT", [H, HPC * HD], bf16, kind="ExternalInput")
    woT = nc.dram_tensor("woT", [HPC * HD, H], bf16, kind="ExternalInput")
    cosd = nc.dram_tensor("cosd", [HD, L], bf16, kind="ExternalInput")
    sinmd = nc.dram_tensor("sinmd", [HD, L], bf16, kind="ExternalInput")
    npat = max(n_patterns, 1)
    maskd = nc.dram_tensor("maskd", [npat, 128, 128], bf16, kind="ExternalInput")
    yT = nc.dram_tensor("yT", [H, L], fp32, kind="ExternalOutput")

    NHC = H // 128  # 16 input-feature blocks

    def qk_phase(tc, w_dram, out_a, wpool, xpool, tpool, pspool, wtag,
                 cos_sb, sinm_sb):
        """Q^T / K^T d-major projection + fused RoPE per (head, chunk)."""
        w_sb = wpool.tile([128, NHC, HPC * HD], bf16, tag="w",
                          name=f"w_{wtag}")
        wr = w_dram[:].rearrange("(a p) m -> p a m", p=128)
        nc.sync.dma_start(out=w_sb[:, 0:4, :], in_=wr[:, 0:4, :])
        nc.sync.dma_start(out=w_sb[:, 4:8, :], in_=wr[:, 4:8, :])
        nc.sync.dma_start(out=w_sb[:, 8:12, :], in_=wr[:, 8:12, :])
        nc.sync.dma_start(out=w_sb[:, 12:16, :], in_=wr[:, 12:16, :])
        for j in range(NQC):
            js = slice(j * QC, (j + 1) * QC)
            x_sb = xpool.tile([128, NHC, QC], bf16, tag="xcols",
                              name=f"x_{wtag}{j}")
            xr = xT[:, js].rearrange("(a p) m -> p a m", p=128)
            nc.sync.dma_start(out=x_sb[:, 0:4, :], in_=xr[:, 0:4, :])
            nc.sync.dma_start(out=x_sb[:, 4:8, :], in_=xr[:, 4:8, :])
            nc.sync.dma_start(out=x_sb[:, 8:12, :], in_=xr[:, 8:12, :])
            nc.sync.dma_start(out=x_sb[:, 12:16, :], in_=xr[:, 12:16, :])
            for h in range(HPC):
                ps = pspool.tile([128, QC], fp32, tag="ps_proj")
                for hc in range(NHC):
                    nc.tensor.matmul(
                        ps[:],
                        w_sb[:, hc, h * HD:(h + 1) * HD],
                        x_sb[:, hc, :],
                        start=(hc == 0), stop=(hc == NHC - 1))
                q = out_a[:, h, js]
                nc.scalar.copy(q, ps[:])
                # rotate-half: pure partition swap, done by SBUF->SBUF DMA
                rq = tpool.tile([128, QC], bf16, tag="rotq")
                nc.sync.dma_start(out=rq[0:64, :], in_=out_a[64:128, h, js])
                nc.sync.dma_start(out=rq[64:128, :], in_=out_a[0:64, h, js])
                nc.vector.tensor_mul(rq[:], rq[:], sinm_sb[:, js])
                nc.vector.tensor_mul(q, q, cos_sb[:, js])
                nc.vector.tensor_add(q, q, rq[:])

    def v_phase(tc, w_dram, va, wpool, xpool, pspool):
        """V pos-major projection (x chunks 256 wide to fit SBUF)."""
        w_sb = wpool.tile([128, NHC, HPC * HD], bf16, tag="w", name="w_v")
        nc.sync.dma_start(
            out=w_sb[:], in_=w_dram[:].rearrange("(a p) m -> p a m", p=128))
        VC = 256
        for j in range(L // VC):
            x_sb = xpool.tile([128, NHC, VC], bf16, tag="xv", name=f"xv{j}")
            nc.sync.dma_start(
                out=x_sb[:],
                in_=xT[:, j * VC:(j + 1) * VC].rearrange(
                    "(a p) m -> p a m", p=128))
            for pb in range(VC // 128):
                psd = [pspool.tile([128, QC], fp32, tag="ps_proj",
                                   name=f"psv{j}_{pb}_{dc}")
                       for dc in range(2)]
                for hc in range(NHC):
                    for dc in range(2):
                        nc.tensor.matmul(
                            psd[dc][:],
                            x_sb[:, hc, pb * 128:(pb + 1) * 128],
                            w_sb[:, hc, dc * QC:(dc + 1) * QC],
                            start=(hc == 0), stop=(hc == NHC - 1))
                for dc in range(2):
                    nc.scalar.copy(
                        va[:, j * (VC // 128) + pb, dc * QC:(dc + 1) * QC],
                        psd[dc][:])

    with tile.TileContext(nc) as tc:
        with tc.tile_pool(name="persist", bufs=1, side="left") as persist:
            # one combined small-constant tile: [trimask patterns | ones]
            cst = persist.tile([128, npat * 128 + 128], bf16, tag="cst")
            for p in range(n_patterns):
                nc.gpsimd.dma_start(out=cst[:, p * 128:(p + 1) * 128],
                                    in_=maskd[p])
            ones_col = npat * 128
            nc.vector.memset(cst[:, ones_col:ones_col + 128], 1.0)
            onesf = persist.tile([128, 128], fp32, tag="onesf")
            nc.vector.memset(onesf[:], 1.0)
            QTa = persist.tile([HD, HPC, L], bf16, tag="qta")
            KTa = persist.tile([HD, HPC, L], bf16, tag="kta")

            # ---------------- phase A: projections + RoPE ----------------
            # Manual pool lifetimes (non-LIFO): weights/x/rope tables are
            # freed before attention while Va spans V-phase..attention.
            wpool_cm = tc.tile_pool(name="wpool", bufs=2, side="right")
            wpool = wpool_cm.__enter__()
            ropec_cm = tc.tile_pool(name="ropec", bufs=1, side="right")
            ropec = ropec_cm.__enter__()
            psp_cm = tc.tile_pool(name="ps_proj", bufs=3, space="PSUM")
            psp = psp_cm.__enter__()

            cos_sb = ropec.tile([HD, L], bf16, tag="cos")
            sinm_sb = ropec.tile([HD, L], bf16, tag="sinm")
            nc.gpsimd.dma_start(out=cos_sb[:], in_=cosd[:])
            nc.gpsimd.dma_start(out=sinm_sb[:], in_=sinmd[:])

            xv_cm = tc.tile_pool(name="xv", bufs=2, side="right")
            xv = xv_cm.__enter__()
            xqk_cm = tc.tile_pool(name="xqk", bufs=2, side="right")
            xqk = xqk_cm.__enter__()
            tpool_cm = tc.tile_pool(name="tpool", bufs=2, side="right")
            tpool = tpool_cm.__enter__()
            qk_phase(tc, wqT, QTa, wpool, xqk, tpool, psp, "q",
                     cos_sb, sinm_sb)
            qk_phase(tc, wkT, KTa, wpool, xqk, tpool, psp, "k",
                     cos_sb, sinm_sb)
            tpool_cm.__exit__(None, None, None)
            xqk_cm.__exit__(None, None, None)
            vp_cm = tc.tile_pool(name="vp", bufs=1, side="left")
            vp_outer = vp_cm.__enter__()
            Va = vp_outer.tile([128, NKB, HPC * HD], bf16, tag="va")
            v_phase(tc, wvT, Va, wpool, xv, psp)
            xv_cm.__exit__(None, None, None)
            ropec_cm.__exit__(None, None, None)
            wpool_cm.__exit__(None, None, None)
            psp_cm.__exit__(None, None, None)

            # -------- phase B + C under Va's lifetime --------
            _attn_and_out(tc, nc, kind, block_pat, QTa, KTa, Va,
                          cst, ones_col, onesf, woT, yT, fp32, bf16, EXP)
            vp_cm.__exit__(None, None, None)

    nc.compile()
    return nc


def _attn_and_out(tc, nc, kind, block_pat, QTa, KTa, Va, cst, ones_col,
                  onesf, woT, yT, fp32, bf16, EXP):
    ones_sb = cst[:, ones_col:ones_col + 1]
    with tc.tile_pool(name="otp", bufs=1, side="left") as otp, \
         tc.tile_pool(name="wo", bufs=1, side="left") as wop:
        OTa = otp.tile([HD, HPC, L], bf16, tag="ota")
        wo_sb = wop.tile([128, HPC, H], bf16, tag="wo")
        # prefetch Wo during attention
        nc.sync.dma_start(
            out=wo_sb[:], in_=woT[:].rearrange("(a p) m -> p a m", p=128))

        # ---------------- phase B: attention ----------------
        # per-(head, q-chunk) flash loop; softmax normalization broadcast
        # via K=1 PE outer product + ACT copy (keeps gpsimd free)
        with tc.tile_pool(name="pp", bufs=4, side="right") as ppool, \
             tc.tile_pool(name="rr", bufs=2, side="right") as rpool, \
             tc.tile_pool(name="bb", bufs=3, side="right") as bpool, \
             tc.tile_pool(name="ps_s", bufs=3, space="PSUM") as ps_s, \
             tc.tile_pool(name="ps_o", bufs=2, space="PSUM") as ps_o, \
             tc.tile_pool(name="ps_r", bufs=2, space="PSUM") as ps_r:
            for h in range(HPC):
                for j in range(NQC):
                    blocks = []
                    for i in range(NKB):
                        live = [t for t in range(4)
                                if kind[4 * j + t][i] != 0]
                        if live:
                            blocks.append((i, live))
                    if not blocks:
                        continue
                    pso = ps_o.tile([128, QC], fp32, tag="pso",
                                    name=f"pso{h}_{j}")
                    psr = ps_r.tile([128, QC], fp32, tag="psr",
                                    name=f"psr{h}_{j}")
                    last = len(blocks) - 1

                    def emit_s(bi):
                        i, live = blocks[bi]
                        t0, t1 = live[0], live[-1]
                        w0, w1 = t0 * 128, (t1 + 1) * 128
                        pss = ps_s.tile([128, QC], fp32, tag="pss",
                                        name=f"pss{h}_{j}_{i}")
                        nc.tensor.matmul(
                            pss[:, w0:w1],
                            KTa[:, h, i * 128:(i + 1) * 128],
                            QTa[:, h, j * QC + w0:j * QC + w1],
                            start=True, stop=True)
                        P = ppool.tile([128, QC], bf16, tag="p",
                                       name=f"p{h}_{j}_{i}")
                        first = (bi == 0)
                        if w0 > 0 and first:
                            nc.vector.memset(P[:, 0:w0], 0.0)
                        if w1 < QC and first:
                            nc.vector.memset(P[:, w1:QC], 0.0)
                        nc.scalar.activation(P[:, w0:w1], pss[:, w0:w1],
                                             EXP, scale=SCALE)
                        for t in range(t0, t1 + 1):
                            qb = 4 * j + t
                            if kind[qb][i] == 0:
                                nc.vector.memset(
                                    P[:, t * 128:(t + 1) * 128], 0.0)
                            elif kind[qb][i] == 2:
                                pat = block_pat[(qb, i)]
                                nc.vector.tensor_mul(
                                    P[:, t * 128:(t + 1) * 128],
                                    P[:, t * 128:(t + 1) * 128],
                                    cst[:, pat * 128:(pat + 1) * 128])
                        return P, w0, first

                    def emit_ov(bi, P, w0, first):
                        i, live = blocks[bi]
                        m0 = 0 if first else w0
                        nc.tensor.matmul(
                            pso[:, m0:QC],
                            Va[:, i, h * HD:(h + 1) * HD],
                            P[:, m0:QC],
                            start=first, stop=(bi == last))
                        nc.tensor.matmul(
                            psr[0:1, m0:QC], ones_sb, P[:, m0:QC],
                            start=first, stop=(bi == last))

                    # 2-deep software pipeline: keep two S tiles in flight
                    # ahead of their O/rowsum consumers so the PE never
                    # waits on the ACT exp latency
                    LOOK = 2
                    pend = []
                    for bi in range(len(blocks)):
                        pend.append((bi,) + emit_s(bi))
                        if len(pend) > LOOK:
                            b0, P0, w00, f0 = pend.pop(0)
                            emit_ov(b0, P0, w00, f0)
                    for b0, P0, w00, f0 in pend:
                        emit_ov(b0, P0, w00, f0)
                    r_sb = rpool.tile([128, QC], fp32, tag="r",
                                      name=f"r{h}_{j}")
                    nc.vector.reciprocal_approx_fast(
                        out=r_sb[0:1, :], in_=psr[0:1, :])
                    rb_sb = rpool.tile([128, QC], bf16, tag="rb",
                                       name=f"rb{h}_{j}")
                    nc.vector.tensor_copy(rb_sb[0:1, :], r_sb[0:1, :])
                    bc_sb = bpool.tile([128, QC], bf16, tag="bc",
                                       name=f"bc{h}_{j}")
                    nc.gpsimd.partition_broadcast(bc_sb[:], rb_sb[0:1, :])
                    nc.vector.tensor_mul(
                        OTa[:, h, j * QC:(j + 1) * QC], pso[:], bc_sb[:])

        # ---------------- phase C: output projection ----------------
        with tc.tile_pool(name="ysb", bufs=3, side="right") as ypool, \
             tc.tile_pool(name="ps_c", bufs=4, space="PSUM") as ps_c:
            for oc in range(H // 128):
                for j in range(NQC):
                    ps = ps_c.tile([128, QC], fp32, tag="psc")
                    for fc in range(HPC):
                        nc.tensor.matmul(
                            ps[:],
                            wo_sb[:, fc, oc * 128:(oc + 1) * 128],
                            OTa[:, fc, j * QC:(j + 1) * QC],
                            start=(fc == 0), stop=(fc == HPC - 1))
                    y_sb = ypool.tile([128, QC], fp32, tag="y")
                    nc.vector.tensor_copy(y_sb[:], ps[:])
                    nc.sync.dma_start(
                        out=yT[oc * 128:(oc + 1) * 128,
                               j * QC:(j + 1) * QC],
                        in_=y_sb[:])


def _prep_inputs(x, mask, Wq, Wk, Wv, Wo, patterns):
    import ml_dtypes
    bf16 = ml_dtypes.bfloat16

    # RoPE tables, d-major [HD, L]
    inv_freq = 1.0 / (ROPE_BASE ** (np.arange(0, HD, 2, dtype=np.float64)
                                    / HD))
    t = np.arange(L, dtype=np.float64)
    freqs = np.outer(t, inv_freq)                     # [L, HD/2]
    emb = np.concatenate((freqs, freqs), axis=-1)     # [L, HD]
    cos = np.cos(emb).T.astype(np.float32)            # [HD, L]
    sin = np.sin(emb).T.astype(np.float32)
    sinm = sin.copy()
    sinm[0:64] = -sin[0:64]
    cos_b = cos.astype(bf16)
    sinm_b = sinm.astype(bf16)

    npat = max(len(patterns), 1)
    maskd = np.zeros((npat, 128, 128), dtype=bf16)
    for i, p in enumerate(patterns):
        maskd[i] = p.astype(np.float32).astype(bf16)

    in_maps = []
    for c in range(NCORES):
        b, half = c // 2, c % 2
        rows = slice(half * HPC * HD, (half + 1) * HPC * HD)
        in_maps.append({
            "xT": np.ascontiguousarray(x[b].T).astype(bf16),
            "wqT": np.ascontiguousarray(Wq[rows, :].T).astype(bf16),
            "wkT": np.ascontiguousarray(Wk[rows, :].T).astype(bf16),
            "wvT": np.ascontiguousarray(Wv[rows, :].T).astype(bf16),
            "woT": np.ascontiguousarray(Wo[:, rows].T).astype(bf16),
            "cosd": cos_b,
            "sinmd": sinm_b,
            "maskd": maskd,
        })
    return in_maps


def kernel(x, mask, Wq, Wk, Wv, Wo, _trace=False):
    from concourse.bass_utils import run_bass_kernel_spmd

    x = np.asarray(x, dtype=np.float32)
    mask2d = np.asarray(mask, dtype=np.int32).reshape(L, L)
    key = mask2d.tobytes()
    if key not in _cache:
        kind, patterns, block_pat = _analyze_mask(mask2d)
        nc = _build(kind, block_pat, len(patterns))
        _cache[key] = (nc, patterns)
    nc, patterns = _cache[key]

    in_maps = _prep_inputs(x, mask, np.asarray(Wq, np.float32),
                           np.asarray(Wk, np.float32),
                           np.asarray(Wv, np.float32),
                           np.asarray(Wo, np.float32), patterns)
    res = run_bass_kernel_spmd(nc, in_maps, list(range(NCORES)),
                               trace=_trace)
    y = np.empty((B, L, H), dtype=np.float32)
    for b in range(B):
        acc = res.results[2 * b]["yT"].astype(np.float32) + \
              res.results[2 * b + 1]["yT"].astype(np.float32)
        y[b] = acc.T
    if _trace:
        kernel.last_results = res
    return y


if __name__ == "__main__":
    import reference
    inputs = reference.setup_inputs()
    inputs = {k: np.asarray(v) for k, v in inputs.items()}
    out = kernel(**inputs)
    exp = np.asarray(reference.reference(**{k: v for k, v in inputs.items()}))
    err = np.abs(out - exp).max() / np.abs(exp).max()
    print("rel err (absmax):", err)


# revision 31
# speedup vs baseline: 1.1947x; 1.0216x over previous
"""Trainium2 Bass kernel: causal multi-head attention with RoPE.

Model: B=4, L=2048, H=2048, NH=16 heads, head_dim=128.
  q = x @ Wq.T ; k = x @ Wk.T ; v = x @ Wv.T        (per-head split)
  q, k <- RoPE(q, k)
  attn = softmax(mask(q k^T / sqrt(hd)))
  out  = (attn @ v) heads-concat @ Wo.T

Sharding (8 cores): hybrid batch x tensor-parallel.  Core c handles
batch b = c//2 and heads half*8..half*8+7 with half = c%2.  Wq/Wk/Wv are
column-sharded (8 heads per core), Wo row-sharded; each core produces a
partial y[b] and the host sums the two partials per batch (the unshard
step) and concatenates batches.

Per-core dataflow (all SBUF-resident, bf16 inputs / fp32 accumulation):
  phase A: Q^T, K^T  [128d x 2048pos] per head (d-major) and V
           [128pos x 1024d] pos-major, via PE matmuls; RoPE on Q^T/K^T
           (rotate-half partition shuffle via SBUF->SBUF DMA, the
           elementwise part on DVE).
  phase B: flash-style causal attention per (head, 512-wide q chunk):
           S^T tile = K_blk^T Q_chunk (PE), P = exp(S^T/sqrt(d)) (ACT),
           block-sparse causal structure with a triangular-mask multiply
           on diagonal blocks (DVE), O^T += V_blk P (PE), rowsum via
           ones-matmul (PE), reciprocal+broadcast+scale for the softmax
           normalization (DVE + GPSIMD).
  phase C: y^T partial = Wo_shard O^T (PE) -> DRAM fp32.
"""

import math
import numpy as np

B, L, H, NH, HD = 4, 2048, 2048, 16, 128
ROPE_BASE = 10000.0
NCORES = 8
HPC = 8          # heads per core
QC = 512         # q chunk width
NQC = L // QC    # 4 q chunks
NKB = L // 128   # 16 kp blocks
SCALE = 1.0 / math.sqrt(HD)

_cache = {}


def _analyze_mask(mask2d):
    """Classify each (q_block, kp_block) 128x128 block of the [L, L] mask.

    Returns (block_kind[16][16] with 0=empty,1=full,2=mixed, patterns,
    pattern_idx dict keyed by block coords). mask2d is int32 [L, L],
    rows=q, cols=kp.
    """
    nb = L // 128
    kind = [[0] * nb for _ in range(nb)]
    patterns = []
    pat_key_to_idx = {}
    block_pat = {}
    for qb in range(nb):
        rows = mask2d[qb * 128:(qb + 1) * 128]
        for kb in range(nb):
            blk = rows[:, kb * 128:(kb + 1) * 128]
            s = int(blk.sum())
            if s == 0:
                kind[qb][kb] = 0
            elif s == 128 * 128:
                kind[qb][kb] = 1
            else:
                kind[qb][kb] = 2
                key = blk.tobytes()
                idx = pat_key_to_idx.get(key)
                if idx is None:
                    idx = len(patterns)
                    pat_key_to_idx[key] = idx
                    # stored transposed: S^T tiles are [kp, q]
                    patterns.append(np.ascontiguousarray(blk.T))
                block_pat[(qb, kb)] = idx
    return kind, patterns, block_pat


def _build(kind, block_pat, n_patterns):
    """Build the SPMD bass program (same for all 8 cores)."""
    import concourse.bass as bass
    import concourse.bacc as bacc
    import concourse.mybir as mybir
    import concourse.tile as tile

    fp32 = mybir.dt.float32
    bf16 = mybir.dt.bfloat16
    EXP = mybir.ActivationFunctionType.Exp

    nc = bacc.Bacc("TRN2", target_bir_lowering=False, debug=False)

    xT = nc.dram_tensor("xT", [H, L], bf16, kind="ExternalInput")
    wqT = nc.dram_tensor("wqT", [H, HPC * HD], bf16, kind="ExternalInput")
    wkT = nc.dram_tensor("wkT", [H, HPC * HD], bf16, kind="ExternalInput")
    wvT = nc.dram_tensor("wvT", [H, HPC * HD], bf16, kind="ExternalInput")
    woT = nc.dram_tensor("woT", [HPC * HD, H], bf16, kind="ExternalInput")
    cosd = nc.dram_tensor("cosd", [HD, L], bf16, kind="ExternalInput")
    sinmd = nc.dram_tensor("sinmd", [HD, L], bf16, kind="ExternalInput")
    npat = max(n_patterns, 1)
    maskd = nc.dram_tensor("maskd", [npat, 128, 128], bf16, kind="ExternalInput")
    yT = nc.dram_tensor("yT", [H, L], fp32, kind="ExternalOutput")

    NHC = H // 128  # 16 input-feature blocks

    def qk_phase(tc, w_dram, out_a, wpool, xpool, tpool, pspool, wtag,
                 cos_sb, sinm_sb):
        """Q^T / K^T d-major projection + fused RoPE per (head, chunk)."""
        w_sb = wpool.tile([128, NHC, HPC * HD], bf16, tag="w",
                          name=f"w_{wtag}")
        wr = w_dram[:].rearrange("(a p) m -> p a m", p=128)
        nc.sync.dma_start(out=w_sb[:, 0:4, :], in_=wr[:, 0:4, :])
        nc.sync.dma_start(out=w_sb[:, 4:8, :], in_=wr[:, 4:8, :])
        nc.sync.dma_start(out=w_sb[:, 8:12, :], in_=wr[:, 8:12, :])
        nc.sync.dma_start(out=w_sb[:, 12:16, :], in_=wr[:, 12:16, :])
        for j in range(NQC):
            js = slice(j * QC, (j + 1) * QC)
            x_sb = xpool.tile([128, NHC, QC], bf16, tag="xcols",
                              name=f"x_{wtag}{j}")
            xr = xT[:, js].rearrange("(a p) m -> p a m", p=128)
            nc.sync.dma_start(out=x_sb[:, 0:4, :], in_=xr[:, 0:4, :])
            nc.sync.dma_start(out=x_sb[:, 4:8, :], in_=xr[:, 4:8, :])
            nc.sync.dma_start(out=x_sb[:, 8:12, :], in_=xr[:, 8:12, :])
            nc.sync.dma_start(out=x_sb[:, 12:16, :], in_=xr[:, 12:16, :])
            for h in range(HPC):
                ps = pspool.tile([128, QC], fp32, tag="ps_proj")
                for hc in range(NHC):
                    nc.tensor.matmul(
                        ps[:],
                        w_sb[:, hc, h * HD:(h + 1) * HD],
                        x_sb[:, hc, :],
                        start=(hc == 0), stop=(hc == NHC - 1))
                q = out_a[:, h, js]
                nc.scalar.copy(q, ps[:])
                # rotate-half: pure partition swap, done by SBUF->SBUF DMA
                rq = tpool.tile([128, QC], bf16, tag="rotq")
                nc.sync.dma_start(out=rq[0:64, :], in_=out_a[64:128, h, js])
                nc.sync.dma_start(out=rq[64:128, :], in_=out_a[0:64, h, js])
                nc.vector.tensor_mul(rq[:], rq[:], sinm_sb[:, js])
                nc.vector.tensor_mul(q, q, cos_sb[:, js])
                nc.vector.tensor_add(q, q, rq[:])

    def v_phase(tc, w_dram, va, wpool, xpool, pspool):
        """V pos-major projection (x chunks 256 wide to fit SBUF)."""
        w_sb = wpool.tile([128, NHC, HPC * HD], bf16, tag="w", name="w_v")
        nc.sync.dma_start(
            out=w_sb[:], in_=w_dram[:].rearrange("(a p) m -> p a m", p=128))
        VC = 256
        for j in range(L // VC):
            x_sb = xpool.tile([128, NHC, VC], bf16, tag="xv", name=f"xv{j}")
            nc.sync.dma_start(
                out=x_sb[:],
                in_=xT[:, j * VC:(j + 1) * VC].rearrange(
                    "(a p) m -> p a m", p=128))
            for pb in range(VC // 128):
                psd = [pspool.tile([128, QC], fp32, tag="ps_proj",
                                   name=f"psv{j}_{pb}_{dc}")
                       for dc in range(2)]
                for hc in range(NHC):
                    for dc in range(2):
                        nc.tensor.matmul(
                            psd[dc][:],
                            x_sb[:, hc, pb * 128:(pb + 1) * 128],
                            w_sb[:, hc, dc * QC:(dc + 1) * QC],
                            start=(hc == 0), stop=(hc == NHC - 1))
                for dc in range(2):
                    nc.scalar.copy(
                        va[:, j * (VC // 128) + pb, dc * QC:(dc + 1) * QC],
                        psd[dc][:])

    with tile.TileContext(nc) as tc:
        with tc.tile_pool(name="persist", bufs=1, side="left") as persist:
            # one combined small-constant tile: [trimask patterns | ones]
            cst = persist.tile([128, npat * 128 + 128], bf16, tag="cst")
            for p in range(n_patterns):
                nc.gpsimd.dma_start(out=cst[:, p * 128:(p + 1) * 128],
                                    in_=maskd[p])
            ones_col = npat * 128
            nc.vector.memset(cst[:, ones_col:ones_col + 128], 1.0)
            onesf = persist.tile([128, 128], fp32, tag="onesf")
            nc.vector.memset(onesf[:], 1.0)
            QTa = persist.tile([HD, HPC, L], bf16, tag="qta")
            KTa = persist.tile([HD, HPC, L], bf16, tag="kta")

            # ---------------- phase A: projections + RoPE ----------------
            # Manual pool lifetimes (non-LIFO): weights/x/rope tables are
            # freed before attention while Va spans V-phase..attention.
            wpool_cm = tc.tile_pool(name="wpool", bufs=2, side="right")
            wpool = wpool_cm.__enter__()
            ropec_cm = tc.tile_pool(name="ropec", bufs=1, side="right")
            ropec = ropec_cm.__enter__()
            psp_cm = tc.tile_pool(name="ps_proj", bufs=3, space="PSUM")
            psp = psp_cm.__enter__()

            cos_sb = ropec.tile([HD, L], bf16, tag="cos")
            sinm_sb = ropec.tile([HD, L], bf16, tag="sinm")
            nc.gpsimd.dma_start(out=cos_sb[:], in_=cosd[:])
            nc.gpsimd.dma_start(out=sinm_sb[:], in_=sinmd[:])

            xv_cm = tc.tile_pool(name="xv", bufs=2, side="right")
            xv = xv_cm.__enter__()
            xqk_cm = tc.tile_pool(name="xqk", bufs=2, side="right")
            xqk = xqk_cm.__enter__()
            tpool_cm = tc.tile_pool(name="tpool", bufs=2, side="right")
            tpool = tpool_cm.__enter__()
            qk_phase(tc, wqT, QTa, wpool, xqk, tpool, psp, "q",
                     cos_sb, sinm_sb)
            qk_phase(tc, wkT, KTa, wpool, xqk, tpool, psp, "k",
                     cos_sb, sinm_sb)
            tpool_cm.__exit__(None, None, None)
            xqk_cm.__exit__(None, None, None)
            vp_cm = tc.tile_pool(name="vp", bufs=1, side="left")
            vp_outer = vp_cm.__enter__()
            Va = vp_outer.tile([128, NKB, HPC * HD], bf16, tag="va")
            v_phase(tc, wvT, Va, wpool, xv, psp)
            xv_cm.__exit__(None, None, None)
            ropec_cm.__exit__(None, None, None)
            wpool_cm.__exit__(None, None, None)
            psp_cm.__exit__(None, None, None)

            # -------- phase B + C under Va's lifetime --------
            _attn_and_out(tc, nc, kind, block_pat, QTa, KTa, Va,
                          cst, ones_col, onesf, woT, yT, fp32, bf16, EXP)
            vp_cm.__exit__(None, None, None)

    nc.compile()
    return nc


def _attn_and_out(tc, nc, kind, block_pat, QTa, KTa, Va, cst, ones_col,
                  onesf, woT, yT, fp32, bf16, EXP):
    ones_sb = cst[:, ones_col:ones_col + 1]
    with tc.tile_pool(name="otp", bufs=1, side="left") as otp, \
         tc.tile_pool(name="wo", bufs=1, side="left") as wop:
        OTa = otp.tile([HD, HPC, L], bf16, tag="ota")
        wo_sb = wop.tile([128, HPC, H], bf16, tag="wo")
        # prefetch Wo during attention
        nc.sync.dma_start(
            out=wo_sb[:], in_=woT[:].rearrange("(a p) m -> p a m", p=128))

        # ---------------- phase B: attention ----------------
        # q-chunk PAIRS inside the kp-block loop: S (and O) matmuls for the
        # two chunks sit back-to-back with the same stationary operand
        # (K block / V block), so the weight load amortizes across both.
        # Rowsums stay in plain partition-0 PSUM banks.  One kp-block of
        # lookahead keeps the PE ahead of the ACT exp latency.
        with tc.tile_pool(name="pp", bufs=6, side="right") as ppool, \
             tc.tile_pool(name="rr", bufs=4, side="right") as rpool, \
             tc.tile_pool(name="bb", bufs=4, side="right") as bpool, \
             tc.tile_pool(name="ps_s", bufs=3, space="PSUM") as ps_s, \
             tc.tile_pool(name="ps_o", bufs=1, space="PSUM") as ps_o, \
             tc.tile_pool(name="ps_r", bufs=1, space="PSUM") as ps_r:
            for h in range(HPC):
                for jpair in ((0, 1), (2, 3)):
                    blocks_j = {}
                    first_i = {}
                    last_i = {}
                    for j in jpair:
                        for i in range(NKB):
                            live = [t for t in range(4)
                                    if kind[4 * j + t][i] != 0]
                            if live:
                                blocks_j.setdefault(i, []).append((j, live))
                                if j not in first_i:
                                    first_i[j] = i
                                last_i[j] = i
                    if not first_i:
                        continue
                    pso = {j: ps_o.tile([128, QC], fp32, tag=f"pso{j % 2}",
                                        name=f"pso{h}_{j}")
                           for j in first_i}
                    psr = {j: ps_r.tile([1, QC], fp32, tag=f"psr{j % 2}",
                                        name=f"psr{h}_{j}")
                           for j in first_i}

                    def emit_s(i, j, live):
                        t0, t1 = live[0], live[-1]
                        w0, w1 = t0 * 128, (t1 + 1) * 128
                        pss = ps_s.tile([128, QC], fp32, tag="pss",
                                        name=f"pss{h}_{j}_{i}")
                        nc.tensor.matmul(
                            pss[:, w0:w1],
                            KTa[:, h, i * 128:(i + 1) * 128],
                            QTa[:, h, j * QC + w0:j * QC + w1],
                            start=True, stop=True)
                        P = ppool.tile([128, QC], bf16, tag="p",
                                       name=f"p{h}_{j}_{i}")
                        first = (first_i[j] == i)
                        if w0 > 0 and first:
                            nc.vector.memset(P[:, 0:w0], 0.0)
                        if w1 < QC and first:
                            nc.vector.memset(P[:, w1:QC], 0.0)
                        nc.scalar.activation(P[:, w0:w1], pss[:, w0:w1],
                                             EXP, scale=SCALE)
                        for t in range(t0, t1 + 1):
                            qb = 4 * j + t
                            if kind[qb][i] == 0:
                                nc.vector.memset(
                                    P[:, t * 128:(t + 1) * 128], 0.0)
                            elif kind[qb][i] == 2:
                                pat = block_pat[(qb, i)]
                                nc.vector.tensor_mul(
                                    P[:, t * 128:(t + 1) * 128],
                                    P[:, t * 128:(t + 1) * 128],
                                    cst[:, pat * 128:(pat + 1) * 128])
                        return (j, P, w0, first)

                    def emit_ovr(i, group):
                        # O matmuls first (V stationary shared), then the
                        # rowsums (ones stationary), then any normalize
                        # whose accumulation just completed
                        for j, P, w0, first in group:
                            m0 = 0 if first else w0
                            nc.tensor.matmul(
                                pso[j][:, m0:QC],
                                Va[:, i, h * HD:(h + 1) * HD],
                                P[:, m0:QC],
                                start=first, stop=(last_i[j] == i))
                        for j, P, w0, first in group:
                            m0 = 0 if first else w0
                            nc.tensor.matmul(
                                psr[j][0:1, m0:QC], ones_sb, P[:, m0:QC],
                                start=first, stop=(last_i[j] == i))
                        for j, P, w0, first in group:
                            if last_i[j] != i:
                                continue
                            r_sb = rpool.tile([128, QC], fp32, tag="r",
                                              name=f"r{h}_{j}")
                            nc.vector.reciprocal_approx_fast(
                                out=r_sb[0:1, :], in_=psr[j][0:1, :])
                            rb_sb = rpool.tile([128, QC], bf16, tag="rb",
                                               name=f"rb{h}_{j}")
                            nc.vector.tensor_copy(rb_sb[0:1, :],
                                                  r_sb[0:1, :])
                            bc_sb = bpool.tile([128, QC], bf16, tag="bc",
                                               name=f"bc{h}_{j}")
                            nc.gpsimd.partition_broadcast(bc_sb[:],
                                                          rb_sb[0:1, :])
                            nc.vector.tensor_mul(
                                OTa[:, h, j * QC:(j + 1) * QC],
                                pso[j][:], bc_sb[:])

                    prev = None
                    for i in sorted(blocks_j):
                        cur = (i, [emit_s(i, j, live)
                                   for j, live in blocks_j[i]])
                        if prev is not None:
                            emit_ovr(*prev)
                        prev = cur
                    if prev is not None:
                        emit_ovr(*prev)

        # ---------------- phase C: output projection ----------------
        with tc.tile_pool(name="ysb", bufs=3, side="right") as ypool, \
             tc.tile_pool(name="ps_c", bufs=4, space="PSUM") as ps_c:
            for oc in range(H // 128):
                for j in range(NQC):
                    ps = ps_c.tile([128, QC], fp32, tag="psc")
                    for fc in range(HPC):
                        nc.tensor.matmul(
                            ps[:],
                            wo_sb[:, fc, oc * 128:(oc + 1) * 128],
                            OTa[:, fc, j * QC:(j + 1) * QC],
                            start=(fc == 0), stop=(fc == HPC - 1))
                    y_sb = ypool.tile([128, QC], fp32, tag="y")
                    nc.vector.tensor_copy(y_sb[:], ps[:])
                    nc.sync.dma_start(
                        out=yT[oc * 128:(oc + 1) * 128,
                               j * QC:(j + 1) * QC],
                        in_=y_sb[:])


def _prep_inputs(x, mask, Wq, Wk, Wv, Wo, patterns):
    import ml_dtypes
    bf16 = ml_dtypes.bfloat16

    # RoPE tables, d-major [HD, L]
    inv_freq = 1.0 / (ROPE_BASE ** (np.arange(0, HD, 2, dtype=np.float64)
                                    / HD))
    t = np.arange(L, dtype=np.float64)
    freqs = np.outer(t, inv_freq)                     # [L, HD/2]
    emb = np.concatenate((freqs, freqs), axis=-1)     # [L, HD]
    cos = np.cos(emb).T.astype(np.float32)            # [HD, L]
    sin = np.sin(emb).T.astype(np.float32)
    sinm = sin.copy()
    sinm[0:64] = -sin[0:64]
    cos_b = cos.astype(bf16)
    sinm_b = sinm.astype(bf16)

    npat = max(len(patterns), 1)
    maskd = np.zeros((npat, 128, 128), dtype=bf16)
    for i, p in enumerate(patterns):
        maskd[i] = p.astype(np.float32).astype(bf16)

    in_maps = []
    for c in range(NCORES):
        b, half = c // 2, c % 2
        rows = slice(half * HPC * HD, (half + 1) * HPC * HD)
        in_maps.append({
            "xT": np.ascontiguousarray(x[b].T).astype(bf16),
            "wqT": np.ascontiguousarray(Wq[rows, :].T).astype(bf16),
            "wkT": np.ascontiguousarray(Wk[rows, :].T).astype(bf16),
            "wvT": np.ascontiguousarray(Wv[rows, :].T).astype(bf16),
            "woT": np.ascontiguousarray(Wo[:, rows].T).astype(bf16),
            "cosd": cos_b,
            "sinmd": sinm_b,
            "maskd": maskd,
        })
    return in_maps


def kernel(x, mask, Wq, Wk, Wv, Wo, _trace=False):
    from concourse.bass_utils import run_bass_kernel_spmd

    x = np.asarray(x, dtype=np.float32)
    mask2d = np.asarray(mask, dtype=np.int32).reshape(L, L)
    key = mask2d.tobytes()
    if key not in _cache:
        kind, patterns, block_pat = _analyze_mask(mask2d)
        nc = _build(kind, block_pat, len(patterns))
        _cache[key] = (nc, patterns)
    nc, patterns = _cache[key]

    in_maps = _prep_inputs(x, mask, np.asarray(Wq, np.float32),
                           np.asarray(Wk, np.float32),
                           np.asarray(Wv, np.float32),
                           np.asarray(Wo, np.float32), patterns)
    res = run_bass_kernel_spmd(nc, in_maps, list(range(NCORES)),
                               trace=_trace)
    y = np.empty((B, L, H), dtype=np.float32)
    for b in range(B):
        acc = res.results[2 * b]["yT"].astype(np.float32) + \
              res.results[2 * b + 1]["yT"].astype(np.float32)
        y[b] = acc.T
    if _trace:
        kernel.last_results = res
    return y


if __name__ == "__main__":
    import reference
    inputs = reference.setup_inputs()
    inputs = {k: np.asarray(v) for k, v in inputs.items()}
    out = kernel(**inputs)
    exp = np.asarray(reference.reference(**{k: v for k, v in inputs.items()}))
    err = np.abs(out - exp).max() / np.abs(exp).max()
    print("rel err (absmax):", err)


# revision 32
# speedup vs baseline: 1.1978x; 1.0025x over previous
"""Trainium2 Bass kernel: causal multi-head attention with RoPE.

Model: B=4, L=2048, H=2048, NH=16 heads, head_dim=128.
  q = x @ Wq.T ; k = x @ Wk.T ; v = x @ Wv.T        (per-head split)
  q, k <- RoPE(q, k)
  attn = softmax(mask(q k^T / sqrt(hd)))
  out  = (attn @ v) heads-concat @ Wo.T

Sharding (8 cores): hybrid batch x tensor-parallel.  Core c handles
batch b = c//2 and heads half*8..half*8+7 with half = c%2.  Wq/Wk/Wv are
column-sharded (8 heads per core), Wo row-sharded; each core produces a
partial y[b] and the host sums the two partials per batch (the unshard
step) and concatenates batches.

Per-core dataflow (all SBUF-resident, bf16 inputs / fp32 accumulation):
  phase A: Q^T, K^T  [128d x 2048pos] per head (d-major) and V
           [128pos x 1024d] pos-major, via PE matmuls; RoPE on Q^T/K^T
           (rotate-half partition shuffle via SBUF->SBUF DMA, the
           elementwise part on DVE).
  phase B: flash-style causal attention per (head, 512-wide q chunk):
           S^T tile = K_blk^T Q_chunk (PE), P = exp(S^T/sqrt(d)) (ACT),
           block-sparse causal structure with a triangular-mask multiply
           on diagonal blocks (DVE), O^T += V_blk P (PE), rowsum via
           ones-matmul (PE), reciprocal+broadcast+scale for the softmax
           normalization (DVE + GPSIMD).
  phase C: y^T partial = Wo_shard O^T (PE) -> DRAM fp32.
"""

import math
import numpy as np

B, L, H, NH, HD = 4, 2048, 2048, 16, 128
ROPE_BASE = 10000.0
NCORES = 8
HPC = 8          # heads per core
QC = 512         # q chunk width
NQC = L // QC    # 4 q chunks
NKB = L // 128   # 16 kp blocks
SCALE = 1.0 / math.sqrt(HD)

_cache = {}


def _analyze_mask(mask2d):
    """Classify each (q_block, kp_block) 128x128 block of the [L, L] mask.

    Returns (block_kind[16][16] with 0=empty,1=full,2=mixed, patterns,
    pattern_idx dict keyed by block coords). mask2d is int32 [L, L],
    rows=q, cols=kp.
    """
    nb = L // 128
    kind = [[0] * nb for _ in range(nb)]
    patterns = []
    pat_key_to_idx = {}
    block_pat = {}
    for qb in range(nb):
        rows = mask2d[qb * 128:(qb + 1) * 128]
        for kb in range(nb):
            blk = rows[:, kb * 128:(kb + 1) * 128]
            s = int(blk.sum())
            if s == 0:
                kind[qb][kb] = 0
            elif s == 128 * 128:
                kind[qb][kb] = 1
            else:
                kind[qb][kb] = 2
                key = blk.tobytes()
                idx = pat_key_to_idx.get(key)
                if idx is None:
                    idx = len(patterns)
                    pat_key_to_idx[key] = idx
                    # stored transposed: S^T tiles are [kp, q]
                    patterns.append(np.ascontiguousarray(blk.T))
                block_pat[(qb, kb)] = idx
    return kind, patterns, block_pat


def _build(kind, block_pat, n_patterns):
    """Build the SPMD bass program (same for all 8 cores)."""
    import concourse.bass as bass
    import concourse.bacc as bacc
    import concourse.mybir as mybir
    import concourse.tile as tile

    fp32 = mybir.dt.float32
    bf16 = mybir.dt.bfloat16
    EXP = mybir.ActivationFunctionType.Exp

    nc = bacc.Bacc("TRN2", target_bir_lowering=False, debug=False)

    xT = nc.dram_tensor("xT", [H, L], bf16, kind="ExternalInput")
    wqT = nc.dram_tensor("wqT", [H, HPC * HD], bf16, kind="ExternalInput")
    wkT = nc.dram_tensor("wkT", [H, HPC * HD], bf16, kind="ExternalInput")
    wvT = nc.dram_tensor("wvT", [H, HPC * HD], bf16, kind="ExternalInput")
    woT = nc.dram_tensor("woT", [HPC * HD, H], bf16, kind="ExternalInput")
    cosd = nc.dram_tensor("cosd", [HD, L], bf16, kind="ExternalInput")
    sinmd = nc.dram_tensor("sinmd", [HD, L], bf16, kind="ExternalInput")
    npat = max(n_patterns, 1)
    maskd = nc.dram_tensor("maskd", [npat, 128, 128], bf16, kind="ExternalInput")
    yT = nc.dram_tensor("yT", [H, L], fp32, kind="ExternalOutput")

    NHC = H // 128  # 16 input-feature blocks

    def qk_phase(tc, w_dram, out_a, wpool, xpool, tpool, pspool, wtag,
                 cos_sb, sinm_sb):
        """Q^T / K^T d-major projection + fused RoPE per (head, chunk)."""
        w_sb = wpool.tile([128, NHC, HPC * HD], bf16, tag="w",
                          name=f"w_{wtag}")
        wr = w_dram[:].rearrange("(a p) m -> p a m", p=128)
        nc.sync.dma_start(out=w_sb[:, 0:4, :], in_=wr[:, 0:4, :])
        nc.sync.dma_start(out=w_sb[:, 4:8, :], in_=wr[:, 4:8, :])
        nc.sync.dma_start(out=w_sb[:, 8:12, :], in_=wr[:, 8:12, :])
        nc.sync.dma_start(out=w_sb[:, 12:16, :], in_=wr[:, 12:16, :])
        for j in range(NQC):
            js = slice(j * QC, (j + 1) * QC)
            x_sb = xpool.tile([128, NHC, QC], bf16, tag="xcols",
                              name=f"x_{wtag}{j}")
            xr = xT[:, js].rearrange("(a p) m -> p a m", p=128)
            nc.sync.dma_start(out=x_sb[:, 0:4, :], in_=xr[:, 0:4, :])
            nc.sync.dma_start(out=x_sb[:, 4:8, :], in_=xr[:, 4:8, :])
            nc.sync.dma_start(out=x_sb[:, 8:12, :], in_=xr[:, 8:12, :])
            nc.sync.dma_start(out=x_sb[:, 12:16, :], in_=xr[:, 12:16, :])
            for h in range(HPC):
                ps = pspool.tile([128, QC], fp32, tag="ps_proj")
                for hc in range(NHC):
                    nc.tensor.matmul(
                        ps[:],
                        w_sb[:, hc, h * HD:(h + 1) * HD],
                        x_sb[:, hc, :],
                        start=(hc == 0), stop=(hc == NHC - 1))
                q = out_a[:, h, js]
                nc.scalar.copy(q, ps[:])
                # rotate-half: pure partition swap, done by SBUF->SBUF DMA
                rq = tpool.tile([128, QC], bf16, tag="rotq")
                nc.sync.dma_start(out=rq[0:64, :], in_=out_a[64:128, h, js])
                nc.sync.dma_start(out=rq[64:128, :], in_=out_a[0:64, h, js])
                nc.vector.tensor_mul(rq[:], rq[:], sinm_sb[:, js])
                nc.vector.tensor_mul(q, q, cos_sb[:, js])
                nc.vector.tensor_add(q, q, rq[:])

    def v_phase(tc, w_dram, va, wpool, xpool, pspool):
        """V pos-major projection (x chunks 256 wide to fit SBUF)."""
        w_sb = wpool.tile([128, NHC, HPC * HD], bf16, tag="w", name="w_v")
        nc.sync.dma_start(
            out=w_sb[:], in_=w_dram[:].rearrange("(a p) m -> p a m", p=128))
        VC = 256
        for j in range(L // VC):
            x_sb = xpool.tile([128, NHC, VC], bf16, tag="xv", name=f"xv{j}")
            nc.sync.dma_start(
                out=x_sb[:],
                in_=xT[:, j * VC:(j + 1) * VC].rearrange(
                    "(a p) m -> p a m", p=128))
            for pb in range(VC // 128):
                psd = [pspool.tile([128, QC], fp32, tag="ps_proj",
                                   name=f"psv{j}_{pb}_{dc}")
                       for dc in range(2)]
                for hc in range(NHC):
                    for dc in range(2):
                        nc.tensor.matmul(
                            psd[dc][:],
                            x_sb[:, hc, pb * 128:(pb + 1) * 128],
                            w_sb[:, hc, dc * QC:(dc + 1) * QC],
                            start=(hc == 0), stop=(hc == NHC - 1))
                for dc in range(2):
                    nc.scalar.copy(
                        va[:, j * (VC // 128) + pb, dc * QC:(dc + 1) * QC],
                        psd[dc][:])

    with tile.TileContext(nc) as tc:
        with tc.tile_pool(name="persist", bufs=1, side="left") as persist:
            # one combined small-constant tile: [trimask patterns | ones]
            cst = persist.tile([128, npat * 128 + 128], bf16, tag="cst")
            for p in range(n_patterns):
                nc.gpsimd.dma_start(out=cst[:, p * 128:(p + 1) * 128],
                                    in_=maskd[p])
            ones_col = npat * 128
            nc.vector.memset(cst[:, ones_col:ones_col + 128], 1.0)
            onesf = persist.tile([128, 128], fp32, tag="onesf")
            nc.vector.memset(onesf[:], 1.0)
            QTa = persist.tile([HD, HPC, L], bf16, tag="qta")
            KTa = persist.tile([HD, HPC, L], bf16, tag="kta")

            # ---------------- phase A: projections + RoPE ----------------
            # Manual pool lifetimes (non-LIFO): weights/x/rope tables are
            # freed before attention while Va spans V-phase..attention.
            wpool_cm = tc.tile_pool(name="wpool", bufs=2, side="right")
            wpool = wpool_cm.__enter__()
            ropec_cm = tc.tile_pool(name="ropec", bufs=1, side="right")
            ropec = ropec_cm.__enter__()
            psp_cm = tc.tile_pool(name="ps_proj", bufs=3, space="PSUM")
            psp = psp_cm.__enter__()

            cos_sb = ropec.tile([HD, L], bf16, tag="cos")
            sinm_sb = ropec.tile([HD, L], bf16, tag="sinm")
            nc.gpsimd.dma_start(out=cos_sb[:], in_=cosd[:])
            nc.gpsimd.dma_start(out=sinm_sb[:], in_=sinmd[:])

            xv_cm = tc.tile_pool(name="xv", bufs=2, side="right")
            xv = xv_cm.__enter__()
            xqk_cm = tc.tile_pool(name="xqk", bufs=2, side="right")
            xqk = xqk_cm.__enter__()
            tpool_cm = tc.tile_pool(name="tpool", bufs=2, side="right")
            tpool = tpool_cm.__enter__()
            qk_phase(tc, wqT, QTa, wpool, xqk, tpool, psp, "q",
                     cos_sb, sinm_sb)
            qk_phase(tc, wkT, KTa, wpool, xqk, tpool, psp, "k",
                     cos_sb, sinm_sb)
            tpool_cm.__exit__(None, None, None)
            xqk_cm.__exit__(None, None, None)
            vp_cm = tc.tile_pool(name="vp", bufs=1, side="left")
            vp_outer = vp_cm.__enter__()
            Va = vp_outer.tile([128, NKB, HPC * HD], bf16, tag="va")
            v_phase(tc, wvT, Va, wpool, xv, psp)
            xv_cm.__exit__(None, None, None)
            ropec_cm.__exit__(None, None, None)
            wpool_cm.__exit__(None, None, None)
            psp_cm.__exit__(None, None, None)

            # -------- phase B + C under Va's lifetime --------
            _attn_and_out(tc, nc, kind, block_pat, QTa, KTa, Va,
                          cst, ones_col, onesf, woT, yT, fp32, bf16, EXP)
            vp_cm.__exit__(None, None, None)

    nc.compile()
    return nc


def _attn_and_out(tc, nc, kind, block_pat, QTa, KTa, Va, cst, ones_col,
                  onesf, woT, yT, fp32, bf16, EXP):
    ones_sb = cst[:, ones_col:ones_col + 1]
    with tc.tile_pool(name="otp", bufs=1, side="left") as otp, \
         tc.tile_pool(name="wo", bufs=1, side="left") as wop:
        OTa = otp.tile([HD, HPC, L], bf16, tag="ota")
        wo_sb = wop.tile([128, HPC, H], bf16, tag="wo")
        # prefetch Wo during attention
        nc.sync.dma_start(
            out=wo_sb[:], in_=woT[:].rearrange("(a p) m -> p a m", p=128))

        # ---------------- phase B: attention ----------------
        # q-chunk PAIRS inside the kp-block loop: S (and O) matmuls for the
        # two chunks sit back-to-back with the same stationary operand
        # (K block / V block), so the weight load amortizes across both.
        # Rowsums stay in plain partition-0 PSUM banks.  One kp-block of
        # lookahead keeps the PE ahead of the ACT exp latency.
        with tc.tile_pool(name="pp", bufs=6, side="right") as ppool, \
             tc.tile_pool(name="rr", bufs=4, side="right") as rpool, \
             tc.tile_pool(name="bb", bufs=4, side="right") as bpool, \
             tc.tile_pool(name="ps_s", bufs=4, space="PSUM") as ps_s, \
             tc.tile_pool(name="ps_o", bufs=1, space="PSUM") as ps_o, \
             tc.tile_pool(name="ps_r", bufs=1, space="PSUM") as ps_r:
            for h in range(HPC):
                for jpair in ((0, 1), (2, 3)):
                    blocks_j = {}
                    first_i = {}
                    last_i = {}
                    for j in jpair:
                        for i in range(NKB):
                            live = [t for t in range(4)
                                    if kind[4 * j + t][i] != 0]
                            if live:
                                blocks_j.setdefault(i, []).append((j, live))
                                if j not in first_i:
                                    first_i[j] = i
                                last_i[j] = i
                    if not first_i:
                        continue
                    pso = {j: ps_o.tile([128, QC], fp32, tag=f"pso{j % 2}",
                                        name=f"pso{h}_{j}")
                           for j in first_i}
                    psr = {j: ps_r.tile([1, QC], fp32, tag=f"psr{j % 2}",
                                        name=f"psr{h}_{j}")
                           for j in first_i}

                    def emit_s(i, j, live):
                        t0, t1 = live[0], live[-1]
                        w0, w1 = t0 * 128, (t1 + 1) * 128
                        pss = ps_s.tile([128, QC], fp32, tag="pss",
                                        name=f"pss{h}_{j}_{i}")
                        nc.tensor.matmul(
                            pss[:, w0:w1],
                            KTa[:, h, i * 128:(i + 1) * 128],
                            QTa[:, h, j * QC + w0:j * QC + w1],
                            start=True, stop=True)
                        P = ppool.tile([128, QC], bf16, tag="p",
                                       name=f"p{h}_{j}_{i}")
                        first = (first_i[j] == i)
                        if w0 > 0 and first:
                            nc.vector.memset(P[:, 0:w0], 0.0)
                        if w1 < QC and first:
                            nc.vector.memset(P[:, w1:QC], 0.0)
                        nc.scalar.activation(P[:, w0:w1], pss[:, w0:w1],
                                             EXP, scale=SCALE)
                        for t in range(t0, t1 + 1):
                            qb = 4 * j + t
                            if kind[qb][i] == 0:
                                nc.vector.memset(
                                    P[:, t * 128:(t + 1) * 128], 0.0)
                            elif kind[qb][i] == 2:
                                pat = block_pat[(qb, i)]
                                nc.vector.tensor_mul(
                                    P[:, t * 128:(t + 1) * 128],
                                    P[:, t * 128:(t + 1) * 128],
                                    cst[:, pat * 128:(pat + 1) * 128])
                        return (j, P, w0, first)

                    def emit_ovr(i, group):
                        # O matmuls first (V stationary shared), then the
                        # rowsums (ones stationary), then any normalize
                        # whose accumulation just completed
                        for j, P, w0, first in group:
                            m0 = 0 if first else w0
                            nc.tensor.matmul(
                                pso[j][:, m0:QC],
                                Va[:, i, h * HD:(h + 1) * HD],
                                P[:, m0:QC],
                                start=first, stop=(last_i[j] == i))
                        for j, P, w0, first in group:
                            m0 = 0 if first else w0
                            nc.tensor.matmul(
                                psr[j][0:1, m0:QC], ones_sb, P[:, m0:QC],
                                start=first, stop=(last_i[j] == i))
                        for j, P, w0, first in group:
                            if last_i[j] != i:
                                continue
                            r_sb = rpool.tile([128, QC], fp32, tag="r",
                                              name=f"r{h}_{j}")
                            nc.vector.reciprocal_approx_fast(
                                out=r_sb[0:1, :], in_=psr[j][0:1, :])
                            rb_sb = rpool.tile([128, QC], bf16, tag="rb",
                                               name=f"rb{h}_{j}")
                            nc.vector.tensor_copy(rb_sb[0:1, :],
                                                  r_sb[0:1, :])
                            bc_sb = bpool.tile([128, QC], bf16, tag="bc",
                                               name=f"bc{h}_{j}")
                            nc.gpsimd.partition_broadcast(bc_sb[:],
                                                          rb_sb[0:1, :])
                            nc.vector.tensor_mul(
                                OTa[:, h, j * QC:(j + 1) * QC],
                                pso[j][:], bc_sb[:])

                    prev = None
                    for i in sorted(blocks_j):
                        cur = (i, [emit_s(i, j, live)
                                   for j, live in blocks_j[i]])
                        if prev is not None:
                            emit_ovr(*prev)
                        prev = cur
                    if prev is not None:
                        emit_ovr(*prev)

        # ---------------- phase C: output projection ----------------
        with tc.tile_pool(name="ysb", bufs=3, side="right") as ypool, \
             tc.tile_pool(name="ps_c", bufs=4, space="PSUM") as ps_c:
            for oc in range(H // 128):
                for j in range(NQC):
                    ps = ps_c.tile([128, QC], fp32, tag="psc")
                    for fc in range(HPC):
                        nc.tensor.matmul(
                            ps[:],
                            wo_sb[:, fc, oc * 128:(oc + 1) * 128],
                            OTa[:, fc, j * QC:(j + 1) * QC],
                            start=(fc == 0), stop=(fc == HPC - 1))
                    y_sb = ypool.tile([128, QC], fp32, tag="y")
                    nc.vector.tensor_copy(y_sb[:], ps[:])
                    nc.sync.dma_start(
                        out=yT[oc * 128:(oc + 1) * 128,
                               j * QC:(j + 1) * QC],
                        in_=y_sb[:])


def _prep_inputs(x, mask, Wq, Wk, Wv, Wo, patterns):
    import ml_dtypes
    bf16 = ml_dtypes.bfloat16

    # RoPE tables, d-major [HD, L]
    inv_freq = 1.0 / (ROPE_BASE ** (np.arange(0, HD, 2, dtype=np.float64)
                                    / HD))
    t = np.arange(L, dtype=np.float64)
    freqs = np.outer(t, inv_freq)                     # [L, HD/2]
    emb = np.concatenate((freqs, freqs), axis=-1)     # [L, HD]
    cos = np.cos(emb).T.astype(np.float32)            # [HD, L]
    sin = np.sin(emb).T.astype(np.float32)
    sinm = sin.copy()
    sinm[0:64] = -sin[0:64]
    cos_b = cos.astype(bf16)
    sinm_b = sinm.astype(bf16)

    npat = max(len(patterns), 1)
    maskd = np.zeros((npat, 128, 128), dtype=bf16)
    for i, p in enumerate(patterns):
        maskd[i] = p.astype(np.float32).astype(bf16)

    in_maps = []
    for c in range(NCORES):
        b, half = c // 2, c % 2
        rows = slice(half * HPC * HD, (half + 1) * HPC * HD)
        in_maps.append({
            "xT": np.ascontiguousarray(x[b].T).astype(bf16),
            "wqT": np.ascontiguousarray(Wq[rows, :].T).astype(bf16),
            "wkT": np.ascontiguousarray(Wk[rows, :].T).astype(bf16),
            "wvT": np.ascontiguousarray(Wv[rows, :].T).astype(bf16),
            "woT": np.ascontiguousarray(Wo[:, rows].T).astype(bf16),
            "cosd": cos_b,
            "sinmd": sinm_b,
            "maskd": maskd,
        })
    return in_maps


def kernel(x, mask, Wq, Wk, Wv, Wo, _trace=False):
    from concourse.bass_utils import run_bass_kernel_spmd

    x = np.asarray(x, dtype=np.float32)
    mask2d = np.asarray(mask, dtype=np.int32).reshape(L, L)
    key = mask2d.tobytes()
    if key not in _cache:
        kind, patterns, block_pat = _analyze_mask(mask2d)
        nc = _build(kind, block_pat, len(patterns))
        _cache[key] = (nc, patterns)
    nc, patterns = _cache[key]

    in_maps = _prep_inputs(x, mask, np.asarray(Wq, np.float32),
                           np.asarray(Wk, np.float32),
                           np.asarray(Wv, np.float32),
                           np.asarray(Wo, np.float32), patterns)
    res = run_bass_kernel_spmd(nc, in_maps, list(range(NCORES)),
                               trace=_trace)
    y = np.empty((B, L, H), dtype=np.float32)
    for b in range(B):
        acc = res.results[2 * b]["yT"].astype(np.float32) + \
              res.results[2 * b + 1]["yT"].astype(np.float32)
        y[b] = acc.T
    if _trace:
        kernel.last_results = res
    return y


if __name__ == "__main__":
    import reference
    inputs = reference.setup_inputs()
    inputs = {k: np.asarray(v) for k, v in inputs.items()}
    out = kernel(**inputs)
    exp = np.asarray(reference.reference(**{k: v for k, v in inputs.items()}))
    err = np.abs(out - exp).max() / np.abs(exp).max()
    print("rel err (absmax):", err)


# revision 33
# speedup vs baseline: 1.2081x; 1.0086x over previous
"""Trainium2 Bass kernel: causal multi-head attention with RoPE.

Model: B=4, L=2048, H=2048, NH=16 heads, head_dim=128.
  q = x @ Wq.T ; k = x @ Wk.T ; v = x @ Wv.T        (per-head split)
  q, k <- RoPE(q, k)
  attn = softmax(mask(q k^T / sqrt(hd)))
  out  = (attn @ v) heads-concat @ Wo.T

Sharding (8 cores): hybrid batch x tensor-parallel.  Core c handles
batch b = c//2 and heads half*8..half*8+7 with half = c%2.  Wq/Wk/Wv are
column-sharded (8 heads per core), Wo row-sharded; each core produces a
partial y[b] and the host sums the two partials per batch (the unshard
step) and concatenates batches.

Per-core dataflow (all SBUF-resident, bf16 inputs / fp32 accumulation):
  phase A: Q^T, K^T  [128d x 2048pos] per head (d-major) and V
           [128pos x 1024d] pos-major, via PE matmuls; RoPE on Q^T/K^T
           (rotate-half partition shuffle via SBUF->SBUF DMA, the
           elementwise part on DVE).
  phase B: flash-style causal attention per (head, 512-wide q chunk):
           S^T tile = K_blk^T Q_chunk (PE), P = exp(S^T/sqrt(d)) (ACT),
           block-sparse causal structure with a triangular-mask multiply
           on diagonal blocks (DVE), O^T += V_blk P (PE), rowsum via
           ones-matmul (PE), reciprocal+broadcast+scale for the softmax
           normalization (DVE + GPSIMD).
  phase C: y^T partial = Wo_shard O^T (PE) -> DRAM fp32.
"""

import math
import numpy as np

B, L, H, NH, HD = 4, 2048, 2048, 16, 128
ROPE_BASE = 10000.0
NCORES = 8
HPC = 8          # heads per core
QC = 512         # q chunk width
NQC = L // QC    # 4 q chunks
NKB = L // 128   # 16 kp blocks
SCALE = 1.0 / math.sqrt(HD)

_cache = {}


def _analyze_mask(mask2d):
    """Classify each (q_block, kp_block) 128x128 block of the [L, L] mask.

    Returns (block_kind[16][16] with 0=empty,1=full,2=mixed, patterns,
    pattern_idx dict keyed by block coords). mask2d is int32 [L, L],
    rows=q, cols=kp.
    """
    nb = L // 128
    kind = [[0] * nb for _ in range(nb)]
    patterns = []
    pat_key_to_idx = {}
    block_pat = {}
    for qb in range(nb):
        rows = mask2d[qb * 128:(qb + 1) * 128]
        for kb in range(nb):
            blk = rows[:, kb * 128:(kb + 1) * 128]
            s = int(blk.sum())
            if s == 0:
                kind[qb][kb] = 0
            elif s == 128 * 128:
                kind[qb][kb] = 1
            else:
                kind[qb][kb] = 2
                key = blk.tobytes()
                idx = pat_key_to_idx.get(key)
                if idx is None:
                    idx = len(patterns)
                    pat_key_to_idx[key] = idx
                    # stored transposed: S^T tiles are [kp, q]
                    patterns.append(np.ascontiguousarray(blk.T))
                block_pat[(qb, kb)] = idx
    return kind, patterns, block_pat


def _build(kind, block_pat, n_patterns):
    """Build the SPMD bass program (same for all 8 cores)."""
    import concourse.bass as bass
    import concourse.bacc as bacc
    import concourse.mybir as mybir
    import concourse.tile as tile

    fp32 = mybir.dt.float32
    bf16 = mybir.dt.bfloat16
    EXP = mybir.ActivationFunctionType.Exp

    nc = bacc.Bacc("TRN2", target_bir_lowering=False, debug=False)

    xT = nc.dram_tensor("xT", [H, L], bf16, kind="ExternalInput")
    wqT = nc.dram_tensor("wqT", [H, HPC * HD], bf16, kind="ExternalInput")
    wkT = nc.dram_tensor("wkT", [H, HPC * HD], bf16, kind="ExternalInput")
    wvT = nc.dram_tensor("wvT", [H, HPC * HD], bf16, kind="ExternalInput")
    woT = nc.dram_tensor("woT", [HPC * HD, H], bf16, kind="ExternalInput")
    cosd = nc.dram_tensor("cosd", [HD, L], bf16, kind="ExternalInput")
    sinmd = nc.dram_tensor("sinmd", [HD, L], bf16, kind="ExternalInput")
    npat = max(n_patterns, 1)
    maskd = nc.dram_tensor("maskd", [npat, 128, 128], bf16, kind="ExternalInput")
    yT = nc.dram_tensor("yT", [H, L], fp32, kind="ExternalOutput")

    NHC = H // 128  # 16 input-feature blocks

    def qk_phase(tc, w_dram, out_a, wpool, xpool, tpool, pspool, wtag,
                 cos_sb, sinm_sb):
        """Q^T / K^T d-major projection + fused RoPE per (head, chunk)."""
        w_sb = wpool.tile([128, NHC, HPC * HD], bf16, tag="w",
                          name=f"w_{wtag}")
        wr = w_dram[:].rearrange("(a p) m -> p a m", p=128)
        nc.sync.dma_start(out=w_sb[:, 0:4, :], in_=wr[:, 0:4, :])
        nc.sync.dma_start(out=w_sb[:, 4:8, :], in_=wr[:, 4:8, :])
        nc.sync.dma_start(out=w_sb[:, 8:12, :], in_=wr[:, 8:12, :])
        nc.sync.dma_start(out=w_sb[:, 12:16, :], in_=wr[:, 12:16, :])
        for j in range(NQC):
            js = slice(j * QC, (j + 1) * QC)
            x_sb = xpool.tile([128, NHC, QC], bf16, tag="xcols",
                              name=f"x_{wtag}{j}")
            xr = xT[:, js].rearrange("(a p) m -> p a m", p=128)
            nc.sync.dma_start(out=x_sb[:, 0:4, :], in_=xr[:, 0:4, :])
            nc.sync.dma_start(out=x_sb[:, 4:8, :], in_=xr[:, 4:8, :])
            nc.sync.dma_start(out=x_sb[:, 8:12, :], in_=xr[:, 8:12, :])
            nc.sync.dma_start(out=x_sb[:, 12:16, :], in_=xr[:, 12:16, :])
            for h in range(HPC):
                ps = pspool.tile([128, QC], fp32, tag="ps_proj")
                for hc in range(NHC):
                    nc.tensor.matmul(
                        ps[:],
                        w_sb[:, hc, h * HD:(h + 1) * HD],
                        x_sb[:, hc, :],
                        start=(hc == 0), stop=(hc == NHC - 1))
                q = out_a[:, h, js]
                nc.scalar.copy(q, ps[:])
                # rotate-half: pure partition swap, done by SBUF->SBUF DMA
                rq = tpool.tile([128, QC], bf16, tag="rotq")
                nc.sync.dma_start(out=rq[0:64, :], in_=out_a[64:128, h, js])
                nc.sync.dma_start(out=rq[64:128, :], in_=out_a[0:64, h, js])
                nc.vector.tensor_mul(rq[:], rq[:], sinm_sb[:, js])
                nc.vector.tensor_mul(q, q, cos_sb[:, js])
                nc.vector.tensor_add(q, q, rq[:])

    def v_phase(tc, w_dram, va, wpool, xpool, pspool):
        """V pos-major projection (x chunks 256 wide to fit SBUF)."""
        w_sb = wpool.tile([128, NHC, HPC * HD], bf16, tag="w", name="w_v")
        nc.sync.dma_start(
            out=w_sb[:], in_=w_dram[:].rearrange("(a p) m -> p a m", p=128))
        VC = 256
        for j in range(L // VC):
            x_sb = xpool.tile([128, NHC, VC], bf16, tag="xv", name=f"xv{j}")
            nc.sync.dma_start(
                out=x_sb[:],
                in_=xT[:, j * VC:(j + 1) * VC].rearrange(
                    "(a p) m -> p a m", p=128))
            for pb in range(VC // 128):
                psd = [pspool.tile([128, QC], fp32, tag="ps_proj",
                                   name=f"psv{j}_{pb}_{dc}")
                       for dc in range(2)]
                for hc in range(NHC):
                    for dc in range(2):
                        nc.tensor.matmul(
                            psd[dc][:],
                            x_sb[:, hc, pb * 128:(pb + 1) * 128],
                            w_sb[:, hc, dc * QC:(dc + 1) * QC],
                            start=(hc == 0), stop=(hc == NHC - 1))
                for dc in range(2):
                    nc.scalar.copy(
                        va[:, j * (VC // 128) + pb, dc * QC:(dc + 1) * QC],
                        psd[dc][:])

    with tile.TileContext(nc) as tc:
        with tc.tile_pool(name="persist", bufs=1, side="left") as persist:
            # one combined small-constant tile: [trimask patterns | ones]
            cst = persist.tile([128, npat * 128 + 128], bf16, tag="cst")
            for p in range(n_patterns):
                nc.gpsimd.dma_start(out=cst[:, p * 128:(p + 1) * 128],
                                    in_=maskd[p])
            ones_col = npat * 128
            nc.vector.memset(cst[:, ones_col:ones_col + 128], 1.0)
            onesf = persist.tile([128, 128], fp32, tag="onesf")
            nc.vector.memset(onesf[:], 1.0)
            QTa = persist.tile([HD, HPC, L], bf16, tag="qta")
            KTa = persist.tile([HD, HPC, L], bf16, tag="kta")

            # ---------------- phase A: projections + RoPE ----------------
            # Manual pool lifetimes (non-LIFO): weights/x/rope tables are
            # freed before attention while Va spans V-phase..attention.
            wpool_cm = tc.tile_pool(name="wpool", bufs=2, side="right")
            wpool = wpool_cm.__enter__()
            ropec_cm = tc.tile_pool(name="ropec", bufs=1, side="right")
            ropec = ropec_cm.__enter__()
            psp_cm = tc.tile_pool(name="ps_proj", bufs=3, space="PSUM")
            psp = psp_cm.__enter__()

            cos_sb = ropec.tile([HD, L], bf16, tag="cos")
            sinm_sb = ropec.tile([HD, L], bf16, tag="sinm")
            nc.gpsimd.dma_start(out=cos_sb[:], in_=cosd[:])
            nc.gpsimd.dma_start(out=sinm_sb[:], in_=sinmd[:])

            xv_cm = tc.tile_pool(name="xv", bufs=2, side="right")
            xv = xv_cm.__enter__()
            xqk_cm = tc.tile_pool(name="xqk", bufs=2, side="right")
            xqk = xqk_cm.__enter__()
            tpool_cm = tc.tile_pool(name="tpool", bufs=2, side="right")
            tpool = tpool_cm.__enter__()
            qk_phase(tc, wqT, QTa, wpool, xqk, tpool, psp, "q",
                     cos_sb, sinm_sb)
            qk_phase(tc, wkT, KTa, wpool, xqk, tpool, psp, "k",
                     cos_sb, sinm_sb)
            tpool_cm.__exit__(None, None, None)
            xqk_cm.__exit__(None, None, None)
            vp_cm = tc.tile_pool(name="vp", bufs=1, side="left")
            vp_outer = vp_cm.__enter__()
            Va = vp_outer.tile([128, NKB, HPC * HD], bf16, tag="va")
            v_phase(tc, wvT, Va, wpool, xv, psp)
            xv_cm.__exit__(None, None, None)
            ropec_cm.__exit__(None, None, None)
            wpool_cm.__exit__(None, None, None)
            psp_cm.__exit__(None, None, None)

            # -------- phase B + C under Va's lifetime --------
            _attn_and_out(tc, nc, kind, block_pat, QTa, KTa, Va,
                          cst, ones_col, onesf, woT, yT, fp32, bf16, EXP)
            vp_cm.__exit__(None, None, None)

    nc.compile()
    return nc


def _attn_and_out(tc, nc, kind, block_pat, QTa, KTa, Va, cst, ones_col,
                  onesf, woT, yT, fp32, bf16, EXP):
    ones_sb = cst[:, ones_col:ones_col + 1]
    with tc.tile_pool(name="otp", bufs=1, side="left") as otp, \
         tc.tile_pool(name="wo", bufs=1, side="left") as wop:
        OTa = otp.tile([HD, HPC, L], bf16, tag="ota")
        wo_sb = wop.tile([128, HPC, H], bf16, tag="wo")
        # prefetch Wo during attention
        nc.sync.dma_start(
            out=wo_sb[:], in_=woT[:].rearrange("(a p) m -> p a m", p=128))

        # ---------------- phase B: attention ----------------
        # q-chunk PAIRS inside the kp-block loop: S (and O) matmuls for the
        # two chunks sit back-to-back with the same stationary operand
        # (K block / V block), so the weight load amortizes across both.
        # Rowsums stay in plain partition-0 PSUM banks.  One kp-block of
        # lookahead keeps the PE ahead of the ACT exp latency.
        with tc.tile_pool(name="pp", bufs=6, side="right") as ppool, \
             tc.tile_pool(name="rr", bufs=4, side="right") as rpool, \
             tc.tile_pool(name="bb", bufs=4, side="right") as bpool, \
             tc.tile_pool(name="ps_s", bufs=4, space="PSUM") as ps_s, \
             tc.tile_pool(name="ps_o", bufs=1, space="PSUM") as ps_o, \
             tc.tile_pool(name="ps_r", bufs=1, space="PSUM") as ps_r:
            for h in range(HPC):
                for jpair in ((0, 1), (2, 3)):
                    blocks_j = {}
                    first_i = {}
                    last_i = {}
                    for j in jpair:
                        for i in range(NKB):
                            live = [t for t in range(4)
                                    if kind[4 * j + t][i] != 0]
                            if live:
                                blocks_j.setdefault(i, []).append((j, live))
                                if j not in first_i:
                                    first_i[j] = i
                                last_i[j] = i
                    if not first_i:
                        continue
                    pso = {j: ps_o.tile([128, QC], fp32, tag=f"pso{j % 2}",
                                        name=f"pso{h}_{j}")
                           for j in first_i}
                    psr = {j: ps_r.tile([1, QC], fp32, tag=f"psr{j % 2}",
                                        name=f"psr{h}_{j}")
                           for j in first_i}

                    def emit_s(i, j, live):
                        t0, t1 = live[0], live[-1]
                        w0, w1 = t0 * 128, (t1 + 1) * 128
                        pss = ps_s.tile([128, QC], fp32, tag="pss",
                                        name=f"pss{h}_{j}_{i}")
                        nc.tensor.matmul(
                            pss[:, w0:w1],
                            KTa[:, h, i * 128:(i + 1) * 128],
                            QTa[:, h, j * QC + w0:j * QC + w1],
                            start=True, stop=True)
                        P = ppool.tile([128, QC], bf16, tag="p",
                                       name=f"p{h}_{j}_{i}")
                        first = (first_i[j] == i)
                        if w0 > 0 and first:
                            nc.vector.memset(P[:, 0:w0], 0.0)
                        if w1 < QC and first:
                            nc.vector.memset(P[:, w1:QC], 0.0)
                        nc.scalar.activation(P[:, w0:w1], pss[:, w0:w1],
                                             EXP, scale=SCALE)
                        for t in range(t0, t1 + 1):
                            qb = 4 * j + t
                            if kind[qb][i] == 0:
                                nc.vector.memset(
                                    P[:, t * 128:(t + 1) * 128], 0.0)
                            elif kind[qb][i] == 2:
                                pat = block_pat[(qb, i)]
                                nc.vector.tensor_mul(
                                    P[:, t * 128:(t + 1) * 128],
                                    P[:, t * 128:(t + 1) * 128],
                                    cst[:, pat * 128:(pat + 1) * 128])
                        return (j, P, w0, first)

                    def emit_ovr(i, group):
                        # O matmuls first (V stationary shared), then the
                        # rowsums (ones stationary), then any normalize
                        # whose accumulation just completed
                        for j, P, w0, first in group:
                            m0 = 0 if first else w0
                            nc.tensor.matmul(
                                pso[j][:, m0:QC],
                                Va[:, i, h * HD:(h + 1) * HD],
                                P[:, m0:QC],
                                start=first, stop=(last_i[j] == i))
                        for j, P, w0, first in group:
                            m0 = 0 if first else w0
                            nc.tensor.matmul(
                                psr[j][0:1, m0:QC], ones_sb, P[:, m0:QC],
                                start=first, stop=(last_i[j] == i))
                        for j, P, w0, first in group:
                            if last_i[j] != i:
                                continue
                            r_sb = rpool.tile([128, QC], fp32, tag="r",
                                              name=f"r{h}_{j}")
                            nc.vector.reciprocal_approx_fast(
                                out=r_sb[0:1, :], in_=psr[j][0:1, :])
                            rb_sb = rpool.tile([128, QC], bf16, tag="rb",
                                               name=f"rb{h}_{j}")
                            nc.vector.tensor_copy(rb_sb[0:1, :],
                                                  r_sb[0:1, :])
                            bc_sb = bpool.tile([128, QC], bf16, tag="bc",
                                               name=f"bc{h}_{j}")
                            nc.gpsimd.partition_broadcast(bc_sb[:],
                                                          rb_sb[0:1, :])
                            nc.vector.tensor_mul(
                                OTa[:, h, j * QC:(j + 1) * QC],
                                pso[j][:], bc_sb[:])

                    prev = None
                    for i in sorted(blocks_j):
                        cur = (i, [emit_s(i, j, live)
                                   for j, live in blocks_j[i]])
                        if prev is not None:
                            emit_ovr(*prev)
                        prev = cur
                    if prev is not None:
                        emit_ovr(*prev)

        # ---------------- phase C: output projection ----------------
        with tc.tile_pool(name="ysb", bufs=3, side="right") as ypool, \
             tc.tile_pool(name="ps_c", bufs=4, space="PSUM") as ps_c:
            for j in range(NQC):
                for oc in range(H // 128):
                    ps = ps_c.tile([128, QC], fp32, tag="psc")
                    for fc in range(HPC):
                        nc.tensor.matmul(
                            ps[:],
                            wo_sb[:, fc, oc * 128:(oc + 1) * 128],
                            OTa[:, fc, j * QC:(j + 1) * QC],
                            start=(fc == 0), stop=(fc == HPC - 1))
                    y_sb = ypool.tile([128, QC], fp32, tag="y")
                    nc.vector.tensor_copy(y_sb[:], ps[:])
                    nc.sync.dma_start(
                        out=yT[oc * 128:(oc + 1) * 128,
                               j * QC:(j + 1) * QC],
                        in_=y_sb[:])


def _prep_inputs(x, mask, Wq, Wk, Wv, Wo, patterns):
    import ml_dtypes
    bf16 = ml_dtypes.bfloat16

    # RoPE tables, d-major [HD, L]
    inv_freq = 1.0 / (ROPE_BASE ** (np.arange(0, HD, 2, dtype=np.float64)
                                    / HD))
    t = np.arange(L, dtype=np.float64)
    freqs = np.outer(t, inv_freq)                     # [L, HD/2]
    emb = np.concatenate((freqs, freqs), axis=-1)     # [L, HD]
    cos = np.cos(emb).T.astype(np.float32)            # [HD, L]
    sin = np.sin(emb).T.astype(np.float32)
    sinm = sin.copy()
    sinm[0:64] = -sin[0:64]
    cos_b = cos.astype(bf16)
    sinm_b = sinm.astype(bf16)

    npat = max(len(patterns), 1)
    maskd = np.zeros((npat, 128, 128), dtype=bf16)
    for i, p in enumerate(patterns):
        maskd[i] = p.astype(np.float32).astype(bf16)

    in_maps = []
    for c in range(NCORES):
        b, half = c // 2, c % 2
        rows = slice(half * HPC * HD, (half + 1) * HPC * HD)
        in_maps.append({
            "xT": np.ascontiguousarray(x[b].T).astype(bf16),
            "wqT": np.ascontiguousarray(Wq[rows, :].T).astype(bf16),
            "wkT": np.ascontiguousarray(Wk[rows, :].T).astype(bf16),
            "wvT": np.ascontiguousarray(Wv[rows, :].T).astype(bf16),
            "woT": np.ascontiguousarray(Wo[:, rows].T).astype(bf16),
            "cosd": cos_b,
            "sinmd": sinm_b,
            "maskd": maskd,
        })
    return in_maps


def kernel(x, mask, Wq, Wk, Wv, Wo, _trace=False):
    from concourse.bass_utils import run_bass_kernel_spmd

    x = np.asarray(x, dtype=np.float32)
    mask2d = np.asarray(mask, dtype=np.int32).reshape(L, L)
    key = mask2d.tobytes()
    if key not in _cache:
        kind, patterns, block_pat = _analyze_mask(mask2d)
        nc = _build(kind, block_pat, len(patterns))
        _cache[key] = (nc, patterns)
    nc, patterns = _cache[key]

    in_maps = _prep_inputs(x, mask, np.asarray(Wq, np.float32),
                           np.asarray(Wk, np.float32),
                           np.asarray(Wv, np.float32),
                           np.asarray(Wo, np.float32), patterns)
    res = run_bass_kernel_spmd(nc, in_maps, list(range(NCORES)),
                               trace=_trace)
    y = np.empty((B, L, H), dtype=np.float32)
    for b in range(B):
        acc = res.results[2 * b]["yT"].astype(np.float32) + \
              res.results[2 * b + 1]["yT"].astype(np.float32)
        y[b] = acc.T
    if _trace:
        kernel.last_results = res
    return y


if __name__ == "__main__":
    import reference
    inputs = reference.setup_inputs()
    inputs = {k: np.asarray(v) for k, v in inputs.items()}
    out = kernel(**inputs)
    exp = np.asarray(reference.reference(**{k: v for k, v in inputs.items()}))
    err = np.abs(out - exp).max() / np.abs(exp).max()
    print("rel err (absmax):", err)
